# revision 48
# baseline (speedup 1.0000x reference)
"""BitNet DiT on 8 Trainium2 NeuronCores — data-parallel over batch (2 images/core).

Host: patchify, time-embedding + adaLN modulation vectors, BitNet weight
quantization (ternary * per-tensor scale) -> fp8 upload.
Device: full 12-block DiT forward per core in a single Bass/Tile kernel.
BitNet matmuls run as exact integer arithmetic in bf16 (|values| <= 127,
fp32 accumulate). Attention runs via transposed-logits + ones-column
softmax-denominator trick.

v3 (vs v2):
- quant chains (ssq, rstd, amax, magic rounds) moved to DVE; rstd via
  Newton-Raphson rsqrt (bit trick + 2 iters) -> no Sqrt ACT table loads
  (only 2 table switches/block: exp <-> gelu).
- q/k computed weight-stationary producing [feat, tok] layout directly:
  kills 48 PE transposes + ACT evacs per block. Per-token activation
  scales applied via a broadcast c-row tile (PE column-transpose trick).
- attention softmax normalization batched: 6 heads per PSUM tile, one
  strided reciprocal + one broadcast multiply (was 48 ACT ops/block).
- softmax weights (eT) in bf16: halves SBUF + faster AV LDWEIGHTS.
- phase order tuned so PE never idles >3.4us (HAM stays at 2.4 GHz):
  v-matmuls before q/k, o-quant interleaved with attention tail.
"""
import math
import os
import sys
import numpy as np

sys.path.insert(0, "/opt/trn_rl_repo")

import ml_dtypes  # noqa: E402
import concourse.bass as bass  # noqa: E402
import concourse.mybir as mybir  # noqa: E402
import concourse.tile as tile  # noqa: E402
from concourse import bacc  # noqa: E402
from concourse.bass_utils import run_bass_kernel_spmd  # noqa: E402
from concourse.masks import make_identity  # noqa: E402

F32 = mybir.dt.float32
F32R = mybir.dt.float32r
I32 = mybir.dt.int32
FP8 = mybir.dt.float8e4
BF16 = mybir.dt.bfloat16
AX = mybir.AxisListType
OP = mybir.AluOpType
AF = mybir.ActivationFunctionType

DIM = 768
DEPTH = int(os.environ.get("KERNEL_DEPTH", "12"))
HEADS = 12
HD = 64
PATCH = 16
IMG = 256
CIN = 3
HID = 4 * DIM
EPS = 1e-6
P = 128
T = 512            # tokens per core (2 images x 256)
NT = T // P        # 4 token tiles
NTOK = 256         # tokens per image
KD = DIM // P      # 6
KH = HID // P      # 24
MAGIC = float(np.float32(3 * 2**22))  # 12582912.0 RNE round-to-int magic
RSQRT_SEED = 0x5F375A86

_CACHED = {}


def _mm_chunks(n):
    out = []
    s = 0
    while s < n:
        e = min(s + 512, n)
        out.append((s, e))
        s = e
    return out


def build_program(depth=DEPTH):
    nc = bacc.Bacc("TRN2", target_bir_lowering=False, debug=False, num_devices=8)

    xpT_d = nc.declare_dram_parameter("xpT", [DIM, T], F32, isOutput=False)
    posb_d = nc.declare_dram_parameter("posb", [NTOK, DIM], F32, isOutput=False)
    patchWT_d = nc.declare_dram_parameter("patchWT", [DIM, DIM], F32, isOutput=False)
    headWT_d = nc.declare_dram_parameter("headWT", [DIM, DIM], F32, isOutput=False)
    headb_d = nc.declare_dram_parameter("headb", [1, DIM], F32, isOutput=False)
    wqkv_d = nc.declare_dram_parameter("wqkv", [depth, DIM, 3 * DIM], FP8, isOutput=False)
    wproj_d = nc.declare_dram_parameter("wproj", [depth, DIM, DIM], FP8, isOutput=False)
    wfc1_d = nc.declare_dram_parameter("wfc1", [depth, DIM, HID], FP8, isOutput=False)
    wfc2_d = nc.declare_dram_parameter("wfc2", [depth, HID, DIM], FP8, isOutput=False)
    # modulation vectors: [block, norm(2), part, img(2), A/B(2), 768]
    mods_d = nc.declare_dram_parameter("mods", [depth, 2, P, 2, 2, DIM], F32, isOutput=False)
    wscl_d = nc.declare_dram_parameter("wscl", [1, 4 * depth], F32, isOutput=False)
    out_d = nc.declare_dram_parameter("zout", [T, DIM], F32, isOutput=True)

    with tile.TileContext(nc) as tc:
        from contextlib import ExitStack
        with ExitStack() as _ctx:
            constp = _ctx.enter_context(tc.tile_pool(name="const", bufs=1))
            residp = _ctx.enter_context(tc.tile_pool(name="resid", bufs=1))
            qkp = _ctx.enter_context(tc.tile_pool(name="qk", bufs=2))
            wp = _ctx.enter_context(tc.tile_pool(name="w", bufs=6))
            modp = _ctx.enter_context(tc.tile_pool(name="mod", bufs=2))
            tmp_ = _ctx.enter_context(tc.tile_pool(name="tm", bufs=2))
            gp = _ctx.enter_context(tc.tile_pool(name="g", bufs=3))
            hp = _ctx.enter_context(tc.tile_pool(name="h", bufs=4))
            xqtp = _ctx.enter_context(tc.tile_pool(name="xqt", bufs=4))
            xqop = _ctx.enter_context(tc.tile_pool(name="xqo", bufs=2))
            xq2p = _ctx.enter_context(tc.tile_pool(name="xq2", bufs=2))
            xqgp = _ctx.enter_context(tc.tile_pool(name="xqg", bufs=2))
            xqsp = _ctx.enter_context(tc.tile_pool(name="xqs", bufs=3))
            eTp = _ctx.enter_context(tc.tile_pool(name="eT", bufs=1))
            cbp = _ctx.enter_context(tc.tile_pool(name="cb", bufs=3))
            scp = _ctx.enter_context(tc.tile_pool(name="sc", bufs=64))
            ps_mm = _ctx.enter_context(tc.tile_pool(name="ps_mm", bufs=3, space="PSUM"))
            ps_lt = _ctx.enter_context(tc.tile_pool(name="ps_lt", bufs=2, space="PSUM"))
            ps_oa = _ctx.enter_context(tc.tile_pool(name="ps_oa", bufs=2, space="PSUM"))
            ps_x = _ctx.enter_context(tc.tile_pool(name="ps_x", bufs=1, space="PSUM"))

            idf = constp.tile([P, P], F32)
            make_identity(nc, idf[:])

            # broadcast w_scales/127 to all partitions
            wsrow = constp.tile([1, 4 * depth], F32)
            nc.sync.dma_start(wsrow[:], wscl_d[:])
            wsb = constp.tile([P, 4 * depth], F32)
            nc.gpsimd.partition_broadcast(wsb[:], wsrow[0:1, :])
            pmag = constp.tile([P, 1], F32)
            nc.vector.memset(pmag[:], MAGIC)

            z = residp.tile([P, NT, DIM], F32)
            v_aug = residp.tile([P, NT, HEADS, HD + 1], BF16)
            nc.vector.memset(v_aug[:, :, :, HD], 1.0)
            o_tm = residp.tile([P, NT, DIM], F32)

            # ---------- quant-chain helpers ----------
            # rsqrt seed for doubled input: rsqrt(2*hx) where hx = ms/2
            K2 = RSQRT_SEED - 0x400000

            def rstd_pair(ssq2):
                """[P,n] rstd = 1/sqrt(ssq/DIM+EPS), bit-trick + 2 Newton iters.
                All DVE; batched over the input's columns."""
                n = ssq2.shape[-1]
                hx = scp.tile([P, n], F32, tag="sc", name="hx")
                nc.vector.tensor_scalar(hx[:], ssq2, 0.5 / DIM, EPS / 2,
                                        OP.mult, OP.add)
                yi = scp.tile([P, n], I32, tag="sc", name="yi")
                nc.vector.tensor_scalar(yi[:], hx[:].bitcast(I32), 1, None,
                                        OP.arith_shift_right)
                yn = scp.tile([P, n], I32, tag="sc", name="yn")
                nc.vector.tensor_scalar(yn[:], yi[:], -1, None, OP.bitwise_xor)
                y = scp.tile([P, n], I32, tag="sc", name="y0")
                nc.vector.tensor_scalar(y[:], yn[:], K2 + 1, None, OP.add)
                yf = y[:].bitcast(F32)
                for it in range(1):
                    a = scp.tile([P, n], F32, tag="sc", name=f"nra{it}")
                    nc.vector.tensor_tensor(a[:], yf, yf, OP.mult)
                    b = scp.tile([P, n], F32, tag="sc", name=f"nrb{it}")
                    nc.vector.tensor_tensor(b[:], a[:], hx[:], OP.mult)
                    c = scp.tile([P, n], F32, tag="sc", name=f"nrc{it}")
                    nc.vector.tensor_scalar(c[:], b[:], -1.0, 1.5, OP.mult, OP.add)
                    y2 = scp.tile([P, n], F32, tag="sc", name=f"nry{it}")
                    nc.vector.tensor_tensor(y2[:], yf, c[:], OP.mult)
                    yf = y2[:]
                return yf

            sq_scr = tmp_.tile([P, DIM], F32, tag="tm", name="sqscr")

            def ssq_act(src_ap, sv_col):
                """sum(src^2) per row on ACT (Square table-free), accum into
                the given [P,1] column."""
                nc.scalar.activation(sq_scr[:], src_ap, AF.Square,
                                     accum_out=sv_col)

            def norm_mod(t, mt, rstd_ap, dst):
                """dst = (z[t]*rstd) * modA + modB  (2 DVE ops)."""
                img = t // 2
                nc.vector.scalar_tensor_tensor(dst, z[:, t, :], rstd_ap,
                                               mt[:, img, 0, :], OP.mult, OP.mult)
                nc.vector.tensor_tensor(dst, dst, mt[:, img, 1, :], OP.add)

            def quant_scales(amax, n, ws_idx):
                """(s127[P,n], c[P,n]) from a filled amax tile."""
                rs = scp.tile([P, n], F32, tag="sc", name="rcp")
                nc.vector.reciprocal(rs[:], amax[:, 0:n])
                s127 = scp.tile([P, n], F32, tag="sc", name="s127")
                nc.vector.tensor_scalar_mul(s127[:], rs[:], 127.0)
                c = scp.tile([P, n], F32, tag="sc", name="cc")
                nc.vector.tensor_scalar(c[:], amax[:, 0:n],
                                        wsb[:, ws_idx:ws_idx + 1], None, OP.mult)
                return s127, c

            def quant_pair(src_aps, ws_idx):
                """amax over each source -> batched (s127[P,n], c[P,n])."""
                n = len(src_aps)
                amax = scp.tile([P, n], F32, tag="sc", name="amax")
                for j, src in enumerate(src_aps):
                    nc.vector.tensor_reduce(amax[:, j:j + 1], src, axis=AX.X,
                                            op=OP.max, apply_absolute_value=True)
                return quant_scales(amax, n, ws_idx)

            def round_dma_act(src_ap, s127_col, dst_slice):
                """magic-round on ACT (in-place), unmagic to bf16 on DVE,
                DMA-transpose into dst."""
                nc.scalar.activation(src_ap, src_ap, AF.Identity,
                                     scale=s127_col, bias=pmag[:])
                xq = xqsp.tile([P, DIM], BF16, tag="xqs", name="xqs")
                nc.vector.tensor_scalar(xq[:], src_ap, MAGIC, None, OP.subtract)
                nc.sync.dma_start_transpose(dst_slice, xq[:])

            def round_dma_dve(src_ap, s127_col, dst_slice):
                """magic-round + unmagic both on DVE (for attn phase where
                ACT is busy with exp)."""
                nc.vector.tensor_scalar(src_ap, src_ap, s127_col, MAGIC,
                                        OP.mult, OP.add)
                xq = xqsp.tile([P, DIM], BF16, tag="xqs", name="xqs")
                nc.vector.tensor_scalar(xq[:], src_ap, MAGIC, None, OP.subtract)
                nc.sync.dma_start_transpose(dst_slice, xq[:])

            # ---------------- patch embed ----------------
            posb_sb = wp.tile([P, 2, DIM], F32, tag="w", name="posb_sb")
            nc.sync.dma_start(posb_sb[:], posb_d.rearrange("(a p) d -> p a d", p=P))
            xpT_h = []
            for hf in range(2):
                xh = qkp.tile([P, KD, NTOK], F32, tag="qk", name="xpT")
                nc.sync.dma_start(
                    xh[:], xpT_d[:, hf * NTOK:(hf + 1) * NTOK].rearrange(
                        "(o p) t -> p o t", p=P))
                xpT_h.append(xh)
            pw_pieces = []
            for i in range(3):
                pwp = wp.tile([P, 2, DIM], F32, tag="w", name="pwp")
                nc.gpsimd.dma_start(
                    pwp[:], patchWT_d[i * 2 * P:(i + 1) * 2 * P, :].rearrange(
                        "(o p) d -> p o d", p=P))
                pw_pieces.append(pwp)
            for t in range(NT):
                for (cs, ce) in _mm_chunks(DIM):
                    pt = ps_mm.tile([P, 512], F32, tag="mm", name="pmm")[:, : ce - cs]
                    for k in range(KD):
                        nc.tensor.matmul(pt[:], xpT_h[t // 2][:, k, (t % 2) * P:(t % 2 + 1) * P],
                                         pw_pieces[k // 2][:, k % 2, cs:ce],
                                         start=(k == 0), stop=(k == KD - 1))
                    nc.vector.tensor_tensor(z[:, t, cs:ce], pt[:], posb_sb[:, t % 2, cs:ce], OP.add)

            def load_w(dram, b, kchunks, width, npieces):
                """Stage one linear's transposed fp8 weights as npieces tiles."""
                span = kchunks // npieces
                tiles = []
                for i in range(npieces):
                    wt = wp.tile([P, span, width], FP8, tag="w")
                    nc.gpsimd.dma_start(
                        wt[:],
                        dram[b, i * span * P:(i + 1) * span * P, :].rearrange(
                            "(o p) f -> p o f", p=P))
                    tiles.append(wt)
                return tiles, span

            def load_mods(b_, n_, name):
                mt = modp.tile([P, 2, 2, DIM], F32, tag="mod", name=name)
                nc.gpsimd.dma_start(mt[:], mods_d[b_, n_])
                return mt

            def p1_pair(pi, mt, ws_idx, half_dst, c_out, ssq2=None):
                """norm1/norm2 + quant for tile pair (2pi, 2pi+1); writes the
                transposed quantized half tile [P, KD, NTOK]."""
                if ssq2 is None:
                    ssq2 = scp.tile([P, 2], F32, tag="sc", name="ssq2")
                    for j in range(2):
                        ssq_act(z[:, 2 * pi + j, :], ssq2[:, j:j + 1])
                rst2 = rstd_pair(ssq2[:])
                hs = []
                for j in range(2):
                    h = hp.tile([P, DIM], F32, tag="h")
                    norm_mod(2 * pi + j, mt, rst2[:, j:j + 1], h[:])
                    hs.append(h)
                s127, cpr = quant_pair([h[:] for h in hs], ws_idx)
                for j in range(2):
                    c_out[2 * pi + j] = cpr[:, j:j + 1]
                    round_dma_act(hs[j][:], s127[:, j:j + 1],
                                  half_dst[:, :, j * P:(j + 1) * P])

            def p1_tile(t, mt, ws_idx, half_dst, c_out):
                """single-tile norm+quant (for latency-critical tail tiles)."""
                ssq1 = scp.tile([P, 1], F32, tag="sc", name="ssq1")
                ssq_act(z[:, t, :], ssq1[:])
                rst = rstd_pair(ssq1[:])
                h = hp.tile([P, DIM], F32, tag="h")
                norm_mod(t, mt, rst, h[:])
                s127, cpr = quant_pair([h[:]], ws_idx)
                c_out[t] = cpr[:, 0:1]
                round_dma_act(h[:], s127[:, 0:1],
                              half_dst[:, :, (t % 2) * P:(t % 2 + 1) * P])

            # ---- prologue: phase 1 of block 0 ----
            mt1_nxt = load_mods(0, 0, "mt1")
            xqT_cur = [xqtp.tile([P, KD, NTOK], BF16, tag="xqt", name="xqt0"), xqtp.tile([P, KD, NTOK], BF16, tag="xqt", name="xqt1")]
            c_cur = [None] * NT
            for pi in range(2):
                p1_pair(pi, mt1_nxt, 0, xqT_cur[pi], c_cur)

            wq_next = load_w(wqkv_d, 0, KD, 3 * DIM, 3)
            for b in range(depth):
                xqT_h, c_list = xqT_cur, c_cur
                mt2 = load_mods(b, 1, "mt2")
                wq_tiles, wq_span = wq_next

                with nc.named_scope(f"b{b}_qkv"):
                    q_fm = qkp.tile([P, KD, T], BF16, tag="qk", name="q_fm")
                    k_fm = qkp.tile([P, KD, T], BF16, tag="qk", name="k_fm")
                    cb = cbp.tile([P, T], F32, tag="cb", name="cbb")

                    def build_cb(half):
                        # per-token scales -> [1,256] row via PE transpose,
                        # then broadcast to all partitions.
                        crow_ps = ps_x.tile([1, 2, P], F32, tag="x", name="crow_ps")
                        for j in range(2):
                            nc.tensor.transpose(crow_ps[:, j, :],
                                                c_list[2 * half + j], idf[:])
                        crow = cbp.tile([1, NTOK], F32, tag="cb", name="crow")
                        nc.vector.tensor_copy(
                            crow[:], crow_ps[:].rearrange("a b c -> a (b c)"))
                        nc.gpsimd.partition_broadcast(
                            cb[:, half * NTOK:(half + 1) * NTOK], crow[0:1, :])

                    def v_mm(t):
                        for (cs, ce) in _mm_chunks(DIM):
                            pt = ps_mm.tile([P, 512], F32, tag="mm", name="pmm")[:, : ce - cs]
                            for k in range(KD):
                                wt = wq_tiles[k // wq_span]
                                nc.tensor.matmul(
                                    pt[:], xqT_h[t // 2][:, k, (t % 2) * P:(t % 2 + 1) * P],
                                    wt[:, k % wq_span, 2 * DIM + cs:2 * DIM + ce],
                                    start=(k == 0), stop=(k == KD - 1))
                            nc.scalar.activation(
                                v_aug[:, t, cs // HD:ce // HD, 0:HD], pt[:],
                                AF.Identity, scale=c_list[t])

                    def qk_mm(half):
                        # weight-stationary, psum [feat 128, tok 256]
                        ts0 = half * NTOK
                        for fc in range(12):
                            pt = ps_mm.tile([P, 512], F32, tag="mm",
                                            name="pmm")[:, :NTOK]
                            for k in range(KD):
                                wt = wq_tiles[k // wq_span]
                                nc.tensor.matmul(
                                    pt[:], wt[:, k % wq_span, fc * P:(fc + 1) * P],
                                    xqT_h[half][:, k, :],
                                    start=(k == 0), stop=(k == KD - 1))
                            if fc < 6:
                                nc.vector.scalar_tensor_tensor(
                                    q_fm[:, fc, ts0:ts0 + NTOK], pt[:], 0.125,
                                    cb[:, ts0:ts0 + NTOK], OP.mult, OP.mult)
                            else:
                                nc.vector.tensor_tensor(
                                    k_fm[:, fc - 6, ts0:ts0 + NTOK], pt[:],
                                    cb[:, ts0:ts0 + NTOK], OP.mult)

                    # proj weights + o-quant dst staged up front
                    wp_tiles, wp_span = load_w(wproj_d, b, KD, DIM, 2)
                    xqoT_h = [xqop.tile([P, KD, NTOK], BF16, tag="xqo",
                                        name=f"xqo{i}") for i in range(2)]
                    co_s = [None] * NT

                    def o_quant_pair(img):
                        srcs = [o_tm[:, 2 * img + j, :] for j in range(2)]
                        s127, cpr = quant_pair(srcs, 4 * b + 1)
                        for j in range(2):
                            co_s[2 * img + j] = cpr[:, j:j + 1]
                            round_dma_dve(srcs[j], s127[:, j:j + 1],
                                          xqoT_h[img][:, :, j * P:(j + 1) * P])

                    wf1_tiles, wf1_span = load_w(wfc1_d, b, KD, HID, 3)
                    xq2T_h = [xq2p.tile([P, KD, NTOK], BF16, tag="xq2",
                                        name=f"xq2{i}") for i in range(2)]
                    c3s = [None] * NT

                    def proj_t(t):
                        for (cs, ce) in _mm_chunks(DIM):
                            pt = ps_mm.tile([P, 512], F32, tag="mm", name="pmm")[:, : ce - cs]
                            for k in range(KD):
                                wt = wp_tiles[k // wp_span]
                                nc.tensor.matmul(
                                    pt[:], xqoT_h[t // 2][:, k, (t % 2) * P:(t % 2 + 1) * P],
                                    wt[:, k % wp_span, cs:ce],
                                    start=(k == 0), stop=(k == KD - 1))
                            nc.vector.scalar_tensor_tensor(
                                z[:, t, cs:ce], pt[:], co_s[t], z[:, t, cs:ce],
                                OP.mult, OP.add)

                    def n2_pair(pi):
                        p1_pair(pi, mt2, 4 * b + 2, xq2T_h[pi], c3s,
                                ssq2=n2_ssq[pi])

                    eTgs = {}

                    def lt_grp(img, g):
                        eTg = eTp.tile([P, 6, 2, NTOK], BF16, tag="eT")
                        eTgs[(img, g)] = eTg
                        for h6 in range(6):
                            hh = 6 * g + h6
                            po = (hh % 2) * HD
                            ch = hh // 2
                            lt = ps_lt.tile([P, 2, NTOK], F32, tag="lt")
                            for mt in range(2):
                                nc.tensor.matmul(
                                    lt[:, mt, :],
                                    k_fm[po:po + HD, ch,
                                         img * NTOK + mt * P: img * NTOK + (mt + 1) * P],
                                    q_fm[po:po + HD, ch,
                                         img * NTOK: (img + 1) * NTOK],
                                    start=True, stop=True)
                            nc.scalar.activation(eTg[:, h6], lt[:], AF.Exp)

                    def oa_grp(img, g):
                        eTg = eTgs.pop((img, g))
                        for nt in range(2):
                            oa = ps_oa.tile([P, 6, HD + 1], F32, tag="oa")
                            for h6 in range(6):
                                for mt in range(2):
                                    nc.tensor.matmul(
                                        oa[:, h6, :],
                                        eTg[:, h6, mt, nt * P:(nt + 1) * P],
                                        v_aug[:, img * 2 + mt, 6 * g + h6, :],
                                        start=(mt == 0), stop=(mt == 1))
                            rinv = scp.tile([P, 6], F32, tag="sc", name="rinv")
                            nc.vector.reciprocal(rinv[:], oa[:, :, HD])
                            dst = o_tm[:, img * 2 + nt,
                                       384 * g:384 * (g + 1)].rearrange(
                                           "p (h d) -> p h d", h=6)
                            nc.vector.tensor_tensor(
                                dst, oa[:, :, 0:HD],
                                rinv[:, :, None].broadcast_to([P, 6, HD]),
                                OP.mult)
                        if g == 1:
                            o_quant_pair(img)

                    build_cb(0)
                    v_mm(0)
                    v_mm(1)
                    qk_mm(0)
                    build_cb(1)
                    v_mm(2)
                    v_mm(3)
                    qk_mm(1)
                    for img in range(2):
                        for g in range(2):
                            lt_grp(img, g)
                            oa_grp(img, g)
                    n2_ssq = [None, None]
                    for t in range(NT):
                        proj_t(t)
                        if t % 2 == 0:
                            n2_ssq[t // 2] = scp.tile([P, 2], F32, tag="sc",
                                                      name="ssq2")
                        ssq_act(z[:, t, :], n2_ssq[t // 2][:, t % 2:t % 2 + 1])
                        if t % 2 == 1:
                            n2_pair(t // 2)

                # --- fc1 + gelu + g-quant ---
                wf2_tiles, wf2_span = load_w(wfc2_d, b, KH, DIM, 3)
                xqg = [None] * NT
                c4s = [None] * NT
                gs = [None] * NT

                gams = [None] * NT

                def gquant(t):
                    gh0, gh1 = gs[t]
                    am = gams[t]
                    ac = scp.tile([P, 1], F32, tag="sc", name="amaxc")
                    nc.vector.tensor_tensor(ac[:], am[:, 0:1], am[:, 1:2], OP.max)
                    rs = scp.tile([P, 1], F32, tag="sc", name="rcp")
                    nc.vector.reciprocal(rs[:], ac[:])
                    s127 = scp.tile([P, 1], F32, tag="sc", name="s127")
                    nc.vector.tensor_scalar_mul(s127[:], rs[:], 127.0)
                    c = scp.tile([P, 1], F32, tag="sc", name="cc")
                    nc.vector.tensor_scalar(c[:], ac[:], wsb[:, 4 * b + 3:4 * b + 4],
                                            None, OP.mult)
                    c4s[t] = c
                    xg = xqgp.tile([P, KH, P], BF16, tag="xqg")
                    xqg[t] = xg
                    for i, gh in enumerate((gh0, gh1)):
                        # magic on ACT (Identity, no table switch), unmagic DVE
                        nc.scalar.activation(gh[:], gh[:], AF.Identity,
                                             scale=s127[:], bias=pmag[:])
                        xq = xqsp.tile([P, HID // 2], BF16, tag="xqs", name="xq24s")
                        nc.vector.tensor_scalar(xq[:], gh[:], MAGIC, None, OP.subtract)
                        nc.sync.dma_start_transpose(xg[:, i * 12:(i + 1) * 12, :], xq[:])

                # --- fc1/fc2 interleaved per tile, fc2 fused with next p1 ---
                fuse = b + 1 < depth
                if fuse:
                    mt1_nxt = load_mods(b + 1, 0, "mt1")
                    xqT_cur = [xqtp.tile([P, KD, NTOK], BF16, tag="xqt",
                                          name=f"xqt{i}") for i in range(2)]
                    c_cur = [None] * NT

                def fc1_t(t):
                    gh0 = gp.tile([P, HID // 2], F32, tag="g")
                    gh1 = gp.tile([P, HID // 2], F32, tag="g")
                    gs[t] = (gh0, gh1)
                    gam = scp.tile([P, 2], F32, tag="sc", name="gam")
                    gams[t] = gam
                    for ci, (cs, ce) in enumerate(_mm_chunks(HID)):
                        pt = ps_mm.tile([P, 512], F32, tag="mm", name="pmm")[:, : ce - cs]
                        for k in range(KD):
                            wt = wf1_tiles[k // wf1_span]
                            nc.tensor.matmul(
                                pt[:], xq2T_h[t // 2][:, k, (t % 2) * P:(t % 2 + 1) * P],
                                wt[:, k % wf1_span, cs:ce],
                                start=(k == 0), stop=(k == KD - 1))
                        gh = gh0 if ci < 3 else gh1
                        off = cs - (0 if ci < 3 else HID // 2)
                        nc.scalar.activation(gh[:, off:off + ce - cs], pt[:],
                                             AF.Gelu_apprx_tanh, scale=c3s[t][:])
                        if ci == 2 or ci == 5:
                            nc.vector.tensor_reduce(
                                gam[:, ci // 3:ci // 3 + 1],
                                (gh0 if ci == 2 else gh1)[:], axis=AX.X,
                                op=OP.max, apply_absolute_value=True)

                def fc2_t(t):
                    for (cs, ce) in _mm_chunks(DIM):
                        pt = ps_mm.tile([P, 512], F32, tag="mm", name="pmm")[:, : ce - cs]
                        for k in range(KH):
                            wt = wf2_tiles[k // wf2_span]
                            nc.tensor.matmul(pt[:], xqg[t][:, k, :],
                                             wt[:, k % wf2_span, cs:ce],
                                             start=(k == 0), stop=(k == KH - 1))
                        nc.vector.scalar_tensor_tensor(
                            z[:, t, cs:ce], pt[:], c4s[t][:], z[:, t, cs:ce],
                            OP.mult, OP.add)

                with nc.named_scope(f"b{b}_mlp"):
                    for t in range(NT):
                        fc1_t(t)
                        if t > 0:
                            gquant(t - 1)
                            fc2_t(t - 1)
                        if t == 2 and fuse:
                            p1_pair(0, mt1_nxt, 4 * (b + 1), xqT_cur[0], c_cur)
                        if t == 3 and fuse:
                            wq_next = load_w(wqkv_d, b + 1, KD, 3 * DIM, 3)
                    gquant(NT - 1)
                    if fuse:
                        p1_tile(2, mt1_nxt, 4 * (b + 1), xqT_cur[1], c_cur)
                    fc2_t(NT - 1)
                    if fuse:
                        p1_tile(3, mt1_nxt, 4 * (b + 1), xqT_cur[1], c_cur)

            # ---------------- final norm + head ----------------
            with nc.named_scope("head"):
                hw_pieces = []
                for i in range(3):
                    hwp = wp.tile([P, 2, DIM], F32, tag="w", name="hwp")
                    nc.gpsimd.dma_start(
                        hwp[:], headWT_d[i * 2 * P:(i + 1) * 2 * P, :].rearrange(
                            "(o p) d -> p o d", p=P))
                    hw_pieces.append(hwp)
                hbrow = tmp_.tile([1, DIM], F32, tag="tm", name="hbrow")
                nc.sync.dma_start(hbrow[:], headb_d[:])
                hbb = wp.tile([P, DIM], F32, tag="w", name="hbb")
                nc.gpsimd.partition_broadcast(hbb[:], hbrow[0:1, :])
                rst_cols = []
                for pi in range(2):
                    ssq2 = scp.tile([P, 2], F32, tag="sc", name="ssqh")
                    for j in range(2):
                        ssq_act(z[:, 2 * pi + j, :], ssq2[:, j:j + 1])
                    rst2 = rstd_pair(ssq2[:])
                    rst_cols += [rst2[:, 0:1], rst2[:, 1:2]]
                for t in range(NT):
                    zn = hp.tile([P, DIM], F32, tag="h")
                    nc.vector.tensor_scalar_mul(zn[:], z[:, t, :], rst_cols[t])
                    znT = hp.tile([P, DIM], F32, tag="h")
                    for g0 in range(0, KD, 4):
                        gn = min(4, KD - g0)
                        ptb = ps_lt.tile([P, 512], F32, tag="lt", name="ptb")[:, : gn * P]
                        for j in range(gn):
                            nc.tensor.transpose(ptb[:, j * P:(j + 1) * P],
                                                zn[:, (g0 + j) * P:(g0 + j + 1) * P], idf[:])
                        nc.vector.tensor_copy(znT[:, g0 * P:(g0 + gn) * P], ptb[:])
                    for (cs, ce) in _mm_chunks(DIM):
                        pt = ps_mm.tile([P, 512], F32, tag="mm", name="pmm")[:, : ce - cs]
                        for k in range(KD):
                            nc.tensor.matmul(pt[:], znT[:, k * P:(k + 1) * P],
                                             hw_pieces[k // 2][:, k % 2, cs:ce],
                                             start=(k == 0), stop=(k == KD - 1))
                        ot = tmp_.tile([P, DIM], F32, tag="tm", name="ot")[:, : ce - cs]
                        nc.vector.tensor_tensor(ot[:], pt[:], hbb[:, cs:ce], OP.add)
                        nc.sync.dma_start(out_d[t * P:(t + 1) * P, cs:ce], ot[:])

    nc.compile()
    return nc


# ---------------------------------------------------------------------------
# host-side numerics (numpy, fp32 — matches jax CPU within ~1e-7)

def _gelu_tanh(x):
    x = x.astype(np.float32)
    c = np.float32(math.sqrt(2.0 / math.pi))
    return np.float32(0.5) * x * (np.float32(1.0) +
                                  np.tanh(c * (x + np.float32(0.044715) * x * x * x)))


def _time_embedding(t, t_w1, t_b1, t_w2, t_b2):
    half = DIM // 2
    freqs = np.exp(-np.log(10000.0) * np.arange(half, dtype=np.float32) / (half - 1)).astype(np.float32)
    args = t[:, None].astype(np.float32) * freqs[None, :]
    emb = np.concatenate([np.sin(args), np.cos(args)], axis=-1).astype(np.float32)
    h = _gelu_tanh(emb @ t_w1.T + t_b1)
    return (h @ t_w2.T + t_b2).astype(np.float32)


def _quant_w(w):
    ws = np.float32(np.mean(np.abs(w), dtype=np.float64)) + np.float32(1e-5)
    wq = np.clip(np.round(w.astype(np.float32) / ws), -1.0, 1.0)
    return wq, ws


def _prepare(inputs):
    x = np.asarray(inputs["x"], np.float32)
    t = np.asarray(inputs["t"], np.float32)
    B = x.shape[0]
    n_cores = 8
    per = B // n_cores  # 2
    p = PATCH
    hh = IMG // p

    xp = x.reshape(B, CIN, hh, p, hh, p).transpose(0, 2, 4, 1, 3, 5).reshape(B, hh * hh, CIN * p * p)

    t_emb = _time_embedding(t, inputs["t_w1"], inputs["t_b1"], inputs["t_w2"], inputs["t_b2"])
    silu = (t_emb / (1.0 + np.exp(-t_emb))).astype(np.float32)

    depth = DEPTH
    mods = np.zeros((depth, 2, B, 2, DIM), np.float32)  # [blk, norm, img, A/B, D]
    wscl = np.zeros((4 * depth,), np.float32)
    wq_all, wp_all, wf1_all, wf2_all = [], [], [], []
    for b in range(depth):
        mod = silu @ np.asarray(inputs["blk_ada_w"][b], np.float32).T + np.asarray(
            inputs["blk_ada_b"][b], np.float32)
        sh1, sc1, sh2, sc2 = np.split(mod, 4, axis=-1)
        n1 = np.asarray(inputs["blk_norm1"][b], np.float32)
        n2 = np.asarray(inputs["blk_norm2"][b], np.float32)
        mods[b, 0, :, 0, :] = n1[None, :] * (1.0 + sc1)
        mods[b, 0, :, 1, :] = sh1
        mods[b, 1, :, 0, :] = n2[None, :] * (1.0 + sc2)
        mods[b, 1, :, 1, :] = sh2

        for j, (nm, lst) in enumerate([("blk_qkv", wq_all), ("blk_proj", wp_all),
                                       ("blk_fc1", wf1_all), ("blk_fc2", wf2_all)]):
            wq, ws = _quant_w(np.asarray(inputs[nm][b], np.float32))
            lst.append(np.ascontiguousarray(wq.T).astype(ml_dtypes.float8_e4m3))
            wscl[4 * b + j] = ws / np.float32(127.0)

    wqkv = np.stack(wq_all)
    wproj = np.stack(wp_all)
    wfc1 = np.stack(wf1_all)
    wfc2 = np.stack(wf2_all)

    posb = (np.asarray(inputs["pos_embed"][0], np.float32) +
            np.asarray(inputs["patch_b"], np.float32)[None, :]).astype(np.float32)
    patchWT = np.ascontiguousarray(np.asarray(inputs["patch_w"], np.float32).T)
    norm_w = np.asarray(inputs["norm_w"], np.float32)
    headWT = np.ascontiguousarray(np.asarray(inputs["head_w"], np.float32).T * norm_w[:, None])
    headb = np.asarray(inputs["head_b"], np.float32)[None, :]

    key = ("prog", depth)
    if key not in _CACHED:
        _CACHED[key] = build_program(depth)
    nc = _CACHED[key]

    in_maps = []
    for c in range(n_cores):
        imgs = slice(c * per, (c + 1) * per)
        xpT = np.ascontiguousarray(xp[imgs].reshape(per * hh * hh, CIN * p * p).T)
        in_maps.append(dict(
            xpT=xpT, posb=posb, patchWT=patchWT, headWT=headWT, headb=headb,
            wqkv=wqkv, wproj=wproj, wfc1=wfc1, wfc2=wfc2,
            mods=np.ascontiguousarray(
                np.broadcast_to(mods[:, :, None, imgs], (depth, 2, 128, per, 2, DIM))),
            wscl=wscl[None, :],
        ))

    return nc, in_maps


def _assemble(res, B=16, per=2):
    p = PATCH
    hh = IMG // p
    out = np.zeros((B, CIN, IMG, IMG), np.float32)
    for c in range(B // per):
        zo = res.results[c]["zout"]  # [512, 768]
        for i in range(per):
            zi = zo[i * 256:(i + 1) * 256]
            out[c * per + i] = zi.reshape(hh, hh, CIN, p, p).transpose(2, 0, 3, 1, 4).reshape(CIN, IMG, IMG)
    return out


def kernel(**inputs):
    nc, in_maps = _prepare(inputs)
    res = run_bass_kernel_spmd(nc, in_maps, list(range(len(in_maps))), trace=False)
    return _assemble(res)


# revision 50
# speedup vs baseline: 1.2229x; 1.2229x over previous
"""BitNet DiT on 8 Trainium2 NeuronCores — data-parallel over batch (2 images/core).

Host: patchify, time-embedding + adaLN modulation vectors, BitNet weight
quantization (ternary * per-tensor scale) -> fp8 upload.
Device: full 12-block DiT forward per core in a single Bass/Tile kernel.
BitNet matmuls run as exact integer arithmetic in bf16 (|values| <= 127,
fp32 accumulate). Attention runs via transposed-logits + ones-column
softmax-denominator trick.

v3 (vs v2):
- quant chains (ssq, rstd, amax, magic rounds) moved to DVE; rstd via
  Newton-Raphson rsqrt (bit trick + 2 iters) -> no Sqrt ACT table loads
  (only 2 table switches/block: exp <-> gelu).
- q/k computed weight-stationary producing [feat, tok] layout directly:
  kills 48 PE transposes + ACT evacs per block. Per-token activation
  scales applied via a broadcast c-row tile (PE column-transpose trick).
- attention softmax normalization batched: 6 heads per PSUM tile, one
  strided reciprocal + one broadcast multiply (was 48 ACT ops/block).
- softmax weights (eT) in bf16: halves SBUF + faster AV LDWEIGHTS.
- phase order tuned so PE never idles >3.4us (HAM stays at 2.4 GHz):
  v-matmuls before q/k, o-quant interleaved with attention tail.
"""
import math
import os
import sys
import numpy as np

sys.path.insert(0, "/opt/trn_rl_repo")

import ml_dtypes  # noqa: E402
import concourse.bass as bass  # noqa: E402
import concourse.mybir as mybir  # noqa: E402
import concourse.tile as tile  # noqa: E402
from concourse import bacc  # noqa: E402
from concourse.bass_utils import run_bass_kernel_spmd  # noqa: E402
from concourse.masks import make_identity  # noqa: E402

F32 = mybir.dt.float32
F32R = mybir.dt.float32r
I32 = mybir.dt.int32
FP8 = mybir.dt.float8e4
BF16 = mybir.dt.bfloat16
AX = mybir.AxisListType
OP = mybir.AluOpType
AF = mybir.ActivationFunctionType

DIM = 768
DEPTH = int(os.environ.get("KERNEL_DEPTH", "12"))
HEADS = 12
HD = 64
PATCH = 16
IMG = 256
CIN = 3
HID = 4 * DIM
EPS = 1e-6
P = 128
T = 512            # tokens per core (2 images x 256)
NT = T // P        # 4 token tiles
NTOK = 256         # tokens per image
KD = DIM // P      # 6
KH = HID // P      # 24
MAGIC = float(np.float32(3 * 2**22))  # 12582912.0 RNE round-to-int magic
RSQRT_SEED = 0x5F375A86

_CACHED = {}


def _mm_chunks(n):
    out = []
    s = 0
    while s < n:
        e = min(s + 512, n)
        out.append((s, e))
        s = e
    return out


def build_program(depth=DEPTH):
    nc = bacc.Bacc("TRN2", target_bir_lowering=False, debug=False, num_devices=8)

    xpT_d = nc.declare_dram_parameter("xpT", [DIM, T], F32, isOutput=False)
    posb_d = nc.declare_dram_parameter("posb", [NTOK, DIM], F32, isOutput=False)
    patchWT_d = nc.declare_dram_parameter("patchWT", [DIM, DIM], F32, isOutput=False)
    headWT_d = nc.declare_dram_parameter("headWT", [DIM, DIM], F32, isOutput=False)
    headb_d = nc.declare_dram_parameter("headb", [1, DIM], F32, isOutput=False)
    wqkv_d = nc.declare_dram_parameter("wqkv", [depth, DIM, 3 * DIM], FP8, isOutput=False)
    wproj_d = nc.declare_dram_parameter("wproj", [depth, DIM, DIM], FP8, isOutput=False)
    wfc1_d = nc.declare_dram_parameter("wfc1", [depth, DIM, HID], FP8, isOutput=False)
    wfc2_d = nc.declare_dram_parameter("wfc2", [depth, HID, DIM], FP8, isOutput=False)
    # modulation vectors: [block, norm(2), part, img(2), A/B(2), 768]
    mods_d = nc.declare_dram_parameter("mods", [depth, 2, P, 2, 2, DIM], F32, isOutput=False)
    wscl_d = nc.declare_dram_parameter("wscl", [1, 4 * depth], F32, isOutput=False)
    out_d = nc.declare_dram_parameter("zout", [T, DIM], F32, isOutput=True)

    with tile.TileContext(nc) as tc:
        from contextlib import ExitStack
        with ExitStack() as _ctx:
            constp = _ctx.enter_context(tc.tile_pool(name="const", bufs=1))
            residp = _ctx.enter_context(tc.tile_pool(name="resid", bufs=1))
            qkp = _ctx.enter_context(tc.tile_pool(name="qk", bufs=2))
            wp = _ctx.enter_context(tc.tile_pool(name="w", bufs=6))
            modp = _ctx.enter_context(tc.tile_pool(name="mod", bufs=2))
            tmp_ = _ctx.enter_context(tc.tile_pool(name="tm", bufs=2))
            gp = _ctx.enter_context(tc.tile_pool(name="g", bufs=3))
            hp = _ctx.enter_context(tc.tile_pool(name="h", bufs=4))
            xqtp = _ctx.enter_context(tc.tile_pool(name="xqt", bufs=4))
            xqop = _ctx.enter_context(tc.tile_pool(name="xqo", bufs=2))
            xq2p = _ctx.enter_context(tc.tile_pool(name="xq2", bufs=2))
            xqgp = _ctx.enter_context(tc.tile_pool(name="xqg", bufs=2))
            xqsp = _ctx.enter_context(tc.tile_pool(name="xqs", bufs=3))
            eTp = _ctx.enter_context(tc.tile_pool(name="eT", bufs=1))
            cbp = _ctx.enter_context(tc.tile_pool(name="cb", bufs=3))
            scp = _ctx.enter_context(tc.tile_pool(name="sc", bufs=64))
            ps_mm = _ctx.enter_context(tc.tile_pool(name="ps_mm", bufs=3, space="PSUM"))
            ps_lt = _ctx.enter_context(tc.tile_pool(name="ps_lt", bufs=2, space="PSUM"))
            ps_oa = _ctx.enter_context(tc.tile_pool(name="ps_oa", bufs=2, space="PSUM"))
            ps_x = _ctx.enter_context(tc.tile_pool(name="ps_x", bufs=1, space="PSUM"))

            idf = constp.tile([P, P], F32)
            make_identity(nc, idf[:])

            # broadcast w_scales/127 to all partitions
            wsrow = constp.tile([1, 4 * depth], F32)
            nc.sync.dma_start(wsrow[:], wscl_d[:])
            wsb = constp.tile([P, 4 * depth], F32)
            nc.gpsimd.partition_broadcast(wsb[:], wsrow[0:1, :])
            pmag = constp.tile([P, 1], F32)
            nc.vector.memset(pmag[:], MAGIC)

            z = residp.tile([P, NT, DIM], F32)
            v_aug = residp.tile([P, NT, HEADS, HD + 1], BF16)
            nc.vector.memset(v_aug[:, :, :, HD], 1.0)
            o_tm = residp.tile([P, NT, DIM], F32)

            # ---------- quant-chain helpers ----------
            # rsqrt seed for doubled input: rsqrt(2*hx) where hx = ms/2
            K2 = RSQRT_SEED - 0x400000

            def rstd_pair(ssq2):
                """[P,n] rstd = 1/sqrt(ssq/DIM+EPS), bit-trick + 2 Newton iters.
                All DVE; batched over the input's columns."""
                n = ssq2.shape[-1]
                hx = scp.tile([P, n], F32, tag="sc", name="hx")
                nc.vector.tensor_scalar(hx[:], ssq2, 0.5 / DIM, EPS / 2,
                                        OP.mult, OP.add)
                yi = scp.tile([P, n], I32, tag="sc", name="yi")
                nc.vector.tensor_scalar(yi[:], hx[:].bitcast(I32), 1, None,
                                        OP.arith_shift_right)
                yn = scp.tile([P, n], I32, tag="sc", name="yn")
                nc.vector.tensor_scalar(yn[:], yi[:], -1, None, OP.bitwise_xor)
                y = scp.tile([P, n], I32, tag="sc", name="y0")
                nc.vector.tensor_scalar(y[:], yn[:], K2 + 1, None, OP.add)
                yf = y[:].bitcast(F32)
                for it in range(1):
                    a = scp.tile([P, n], F32, tag="sc", name=f"nra{it}")
                    nc.vector.tensor_tensor(a[:], yf, yf, OP.mult)
                    b = scp.tile([P, n], F32, tag="sc", name=f"nrb{it}")
                    nc.vector.tensor_tensor(b[:], a[:], hx[:], OP.mult)
                    c = scp.tile([P, n], F32, tag="sc", name=f"nrc{it}")
                    nc.vector.tensor_scalar(c[:], b[:], -1.0, 1.5, OP.mult, OP.add)
                    y2 = scp.tile([P, n], F32, tag="sc", name=f"nry{it}")
                    nc.vector.tensor_tensor(y2[:], yf, c[:], OP.mult)
                    yf = y2[:]
                return yf

            sq_scr = tmp_.tile([P, DIM], F32, tag="tm", name="sqscr")

            def ssq_act(src_ap, sv_col):
                """sum(src^2) per row on ACT (Square table-free), accum into
                the given [P,1] column."""
                nc.scalar.activation(sq_scr[:], src_ap, AF.Square,
                                     accum_out=sv_col)

            def norm_mod(t, mt, rstd_ap, dst):
                """dst = (z[t]*rstd) * modA + modB  (2 DVE ops)."""
                img = t // 2
                nc.vector.scalar_tensor_tensor(dst, z[:, t, :], rstd_ap,
                                               mt[:, img, 0, :], OP.mult, OP.mult)
                nc.vector.tensor_tensor(dst, dst, mt[:, img, 1, :], OP.add)

            def quant_scales(amax, n, ws_idx):
                """(s127[P,n], c[P,n]) from a filled amax tile."""
                rs = scp.tile([P, n], F32, tag="sc", name="rcp")
                nc.vector.reciprocal(rs[:], amax[:, 0:n])
                s127 = scp.tile([P, n], F32, tag="sc", name="s127")
                nc.vector.tensor_scalar_mul(s127[:], rs[:], 127.0)
                c = scp.tile([P, n], F32, tag="sc", name="cc")
                nc.vector.tensor_scalar(c[:], amax[:, 0:n],
                                        wsb[:, ws_idx:ws_idx + 1], None, OP.mult)
                return s127, c

            def quant_pair(src_aps, ws_idx):
                """amax over each source -> batched (s127[P,n], c[P,n])."""
                n = len(src_aps)
                amax = scp.tile([P, n], F32, tag="sc", name="amax")
                for j, src in enumerate(src_aps):
                    nc.vector.tensor_reduce(amax[:, j:j + 1], src, axis=AX.X,
                                            op=OP.max, apply_absolute_value=True)
                return quant_scales(amax, n, ws_idx)

            def round_dma_act(src_ap, s127_col, dst_slice):
                """magic-round on ACT (in-place), unmagic to bf16 on DVE,
                DMA-transpose into dst."""
                nc.scalar.activation(src_ap, src_ap, AF.Identity,
                                     scale=s127_col, bias=pmag[:])
                xq = xqsp.tile([P, DIM], BF16, tag="xqs", name="xqs")
                nc.vector.tensor_scalar(xq[:], src_ap, MAGIC, None, OP.subtract)
                nc.sync.dma_start_transpose(dst_slice, xq[:])

            def round_dma_dve(src_ap, s127_col, dst_slice):
                """magic-round + unmagic both on DVE (for attn phase where
                ACT is busy with exp)."""
                nc.vector.tensor_scalar(src_ap, src_ap, s127_col, MAGIC,
                                        OP.mult, OP.add)
                xq = xqsp.tile([P, DIM], BF16, tag="xqs", name="xqs")
                nc.vector.tensor_scalar(xq[:], src_ap, MAGIC, None, OP.subtract)
                nc.sync.dma_start_transpose(dst_slice, xq[:])

            # ---------------- patch embed ----------------
            posb_sb = wp.tile([P, 2, DIM], F32, tag="w", name="posb_sb")
            nc.sync.dma_start(posb_sb[:], posb_d.rearrange("(a p) d -> p a d", p=P))
            xpT_h = []
            for hf in range(2):
                xh = qkp.tile([P, KD, NTOK], F32, tag="qk", name="xpT")
                nc.sync.dma_start(
                    xh[:], xpT_d[:, hf * NTOK:(hf + 1) * NTOK].rearrange(
                        "(o p) t -> p o t", p=P))
                xpT_h.append(xh)
            pw_pieces = []
            for i in range(3):
                pwp = wp.tile([P, 2, DIM], F32, tag="w", name="pwp")
                nc.gpsimd.dma_start(
                    pwp[:], patchWT_d[i * 2 * P:(i + 1) * 2 * P, :].rearrange(
                        "(o p) d -> p o d", p=P))
                pw_pieces.append(pwp)
            for t in range(NT):
                for (cs, ce) in _mm_chunks(DIM):
                    pt = ps_mm.tile([P, 512], F32, tag="mm", name="pmm")[:, : ce - cs]
                    for k in range(KD):
                        nc.tensor.matmul(pt[:], xpT_h[t // 2][:, k, (t % 2) * P:(t % 2 + 1) * P],
                                         pw_pieces[k // 2][:, k % 2, cs:ce],
                                         start=(k == 0), stop=(k == KD - 1))
                    nc.vector.tensor_tensor(z[:, t, cs:ce], pt[:], posb_sb[:, t % 2, cs:ce], OP.add)

            def load_w(dram, b, kchunks, width, npieces):
                """Stage one linear's transposed fp8 weights as npieces tiles."""
                span = kchunks // npieces
                tiles = []
                for i in range(npieces):
                    wt = wp.tile([P, span, width], FP8, tag="w")
                    nc.gpsimd.dma_start(
                        wt[:],
                        dram[b, i * span * P:(i + 1) * span * P, :].rearrange(
                            "(o p) f -> p o f", p=P))
                    tiles.append(wt)
                return tiles, span

            def load_mods(b_, n_, name):
                mt = modp.tile([P, 2, 2, DIM], F32, tag="mod", name=name)
                nc.gpsimd.dma_start(mt[:], mods_d[b_, n_])
                return mt

            def p1_pair(pi, mt, ws_idx, half_dst, c_out, ssq2=None):
                """norm1/norm2 + quant for tile pair (2pi, 2pi+1); writes the
                transposed quantized half tile [P, KD, NTOK]."""
                if ssq2 is None:
                    ssq2 = scp.tile([P, 2], F32, tag="sc", name="ssq2")
                    for j in range(2):
                        ssq_act(z[:, 2 * pi + j, :], ssq2[:, j:j + 1])
                rst2 = rstd_pair(ssq2[:])
                hs = []
                for j in range(2):
                    h = hp.tile([P, DIM], F32, tag="h")
                    norm_mod(2 * pi + j, mt, rst2[:, j:j + 1], h[:])
                    hs.append(h)
                s127, cpr = quant_pair([h[:] for h in hs], ws_idx)
                for j in range(2):
                    c_out[2 * pi + j] = cpr[:, j:j + 1]
                    round_dma_act(hs[j][:], s127[:, j:j + 1],
                                  half_dst[:, :, j * P:(j + 1) * P])

            def p1_tile(t, mt, ws_idx, half_dst, c_out, ssq_split=None):
                """single-tile norm+quant (for latency-critical tail tiles)."""
                ssq1 = scp.tile([P, 1], F32, tag="sc", name="ssq1")
                if ssq_split is None:
                    ssq_act(z[:, t, :], ssq1[:])
                else:
                    nc.vector.tensor_tensor(ssq1[:], ssq_split[:, 0:1],
                                            ssq_split[:, 1:2], OP.add)
                rst = rstd_pair(ssq1[:])
                h = hp.tile([P, DIM], F32, tag="h")
                norm_mod(t, mt, rst, h[:])
                s127, cpr = quant_pair([h[:]], ws_idx)
                c_out[t] = cpr[:, 0:1]
                round_dma_act(h[:], s127[:, 0:1],
                              half_dst[:, :, (t % 2) * P:(t % 2 + 1) * P])

            # ---- prologue: phase 1 of block 0 ----
            mt1_nxt = load_mods(0, 0, "mt1")
            xqT_cur = [xqtp.tile([P, KD, NTOK], BF16, tag="xqt", name="xqt0"), xqtp.tile([P, KD, NTOK], BF16, tag="xqt", name="xqt1")]
            c_cur = [None] * NT
            for pi in range(2):
                p1_pair(pi, mt1_nxt, 0, xqT_cur[pi], c_cur)

            wq_next = load_w(wqkv_d, 0, KD, 3 * DIM, 3)
            for b in range(depth):
                xqT_h, c_list = xqT_cur, c_cur
                mt2 = load_mods(b, 1, "mt2")
                wq_tiles, wq_span = wq_next

                with nc.named_scope(f"b{b}_qkv"):
                    q_fm = qkp.tile([P, KD, T], BF16, tag="qk", name="q_fm")
                    k_fm = qkp.tile([P, KD, T], BF16, tag="qk", name="k_fm")
                    cb = cbp.tile([P, T], F32, tag="cb", name="cbb")

                    def build_cb(half):
                        # per-token scales -> [1,256] row via PE transpose,
                        # then broadcast to all partitions.
                        crow_ps = ps_x.tile([1, 2, P], F32, tag="x", name="crow_ps")
                        for j in range(2):
                            nc.tensor.transpose(crow_ps[:, j, :],
                                                c_list[2 * half + j], idf[:])
                        crow = cbp.tile([1, NTOK], F32, tag="cb", name="crow")
                        nc.vector.tensor_copy(
                            crow[:], crow_ps[:].rearrange("a b c -> a (b c)"))
                        nc.gpsimd.partition_broadcast(
                            cb[:, half * NTOK:(half + 1) * NTOK], crow[0:1, :])

                    def v_mm(t):
                        for (cs, ce) in _mm_chunks(DIM):
                            pt = ps_mm.tile([P, 512], F32, tag="mm", name="pmm")[:, : ce - cs]
                            for k in range(KD):
                                wt = wq_tiles[k // wq_span]
                                nc.tensor.matmul(
                                    pt[:], xqT_h[t // 2][:, k, (t % 2) * P:(t % 2 + 1) * P],
                                    wt[:, k % wq_span, 2 * DIM + cs:2 * DIM + ce],
                                    start=(k == 0), stop=(k == KD - 1))
                            nc.scalar.activation(
                                v_aug[:, t, cs // HD:ce // HD, 0:HD], pt[:],
                                AF.Identity, scale=c_list[t])

                    def qk_mm(half):
                        # weight-stationary, psum [feat 128, tok 256]
                        ts0 = half * NTOK
                        for fc in range(12):
                            pt = ps_mm.tile([P, 512], F32, tag="mm",
                                            name="pmm")[:, :NTOK]
                            for k in range(KD):
                                wt = wq_tiles[k // wq_span]
                                nc.tensor.matmul(
                                    pt[:], wt[:, k % wq_span, fc * P:(fc + 1) * P],
                                    xqT_h[half][:, k, :],
                                    start=(k == 0), stop=(k == KD - 1))
                            if fc < 6:
                                nc.vector.scalar_tensor_tensor(
                                    q_fm[:, fc, ts0:ts0 + NTOK], pt[:], 0.125,
                                    cb[:, ts0:ts0 + NTOK], OP.mult, OP.mult)
                            else:
                                nc.vector.tensor_tensor(
                                    k_fm[:, fc - 6, ts0:ts0 + NTOK], pt[:],
                                    cb[:, ts0:ts0 + NTOK], OP.mult)

                    # proj weights + o-quant dst staged up front
                    wp_tiles, wp_span = load_w(wproj_d, b, KD, DIM, 2)
                    xqoT_h = [xqop.tile([P, KD, NTOK], BF16, tag="xqo",
                                        name=f"xqo{i}") for i in range(2)]
                    co_s = [None] * NT

                    oams = {}

                    def o_quant_pair(img):
                        oam = oams.pop(img)
                        amax = scp.tile([P, 2], F32, tag="sc", name="amax")
                        for j in range(2):
                            nc.vector.tensor_tensor(amax[:, j:j + 1],
                                                    oam[:, j, 0:1],
                                                    oam[:, j, 1:2], OP.max)
                        s127, cpr = quant_scales(amax, 2, 4 * b + 1)
                        for j in range(2):
                            co_s[2 * img + j] = cpr[:, j:j + 1]
                            round_dma_dve(o_tm[:, 2 * img + j, :],
                                          s127[:, j:j + 1],
                                          xqoT_h[img][:, :, j * P:(j + 1) * P])

                    wf1_tiles, wf1_span = load_w(wfc1_d, b, KD, HID, 3)
                    xq2T_h = [xq2p.tile([P, KD, NTOK], BF16, tag="xq2",
                                        name=f"xq2{i}") for i in range(2)]
                    c3s = [None] * NT

                    def proj_t(t):
                        for (cs, ce) in _mm_chunks(DIM):
                            pt = ps_mm.tile([P, 512], F32, tag="mm", name="pmm")[:, : ce - cs]
                            for k in range(KD):
                                wt = wp_tiles[k // wp_span]
                                nc.tensor.matmul(
                                    pt[:], xqoT_h[t // 2][:, k, (t % 2) * P:(t % 2 + 1) * P],
                                    wt[:, k % wp_span, cs:ce],
                                    start=(k == 0), stop=(k == KD - 1))
                            nc.vector.scalar_tensor_tensor(
                                z[:, t, cs:ce], pt[:], co_s[t], z[:, t, cs:ce],
                                OP.mult, OP.add)

                    def n2_pair(pi):
                        p1_pair(pi, mt2, 4 * b + 2, xq2T_h[pi], c3s,
                                ssq2=n2_ssq[pi])

                    eTgs = {}

                    def lt_grp(img, g):
                        eTg = eTp.tile([P, 6, 2, NTOK], BF16, tag="eT")
                        eTgs[(img, g)] = eTg
                        for h6 in range(6):
                            hh = 6 * g + h6
                            po = (hh % 2) * HD
                            ch = hh // 2
                            lt = ps_lt.tile([P, 2, NTOK], F32, tag="lt")
                            for mt in range(2):
                                nc.tensor.matmul(
                                    lt[:, mt, :],
                                    k_fm[po:po + HD, ch,
                                         img * NTOK + mt * P: img * NTOK + (mt + 1) * P],
                                    q_fm[po:po + HD, ch,
                                         img * NTOK: (img + 1) * NTOK],
                                    start=True, stop=True)
                            nc.scalar.activation(eTg[:, h6], lt[:], AF.Exp)

                    def oa_grp(img, g):
                        eTg = eTgs.pop((img, g))
                        for nt in range(2):
                            oa = ps_oa.tile([P, 6, HD + 1], F32, tag="oa")
                            for h6 in range(6):
                                for mt in range(2):
                                    nc.tensor.matmul(
                                        oa[:, h6, :],
                                        eTg[:, h6, mt, nt * P:(nt + 1) * P],
                                        v_aug[:, img * 2 + mt, 6 * g + h6, :],
                                        start=(mt == 0), stop=(mt == 1))
                            rinv = scp.tile([P, 6], F32, tag="sc", name="rinv")
                            nc.vector.reciprocal(rinv[:], oa[:, :, HD])
                            dst = o_tm[:, img * 2 + nt,
                                       384 * g:384 * (g + 1)].rearrange(
                                           "p (h d) -> p h d", h=6)
                            nc.vector.tensor_tensor(
                                dst, oa[:, :, 0:HD],
                                rinv[:, :, None].broadcast_to([P, 6, HD]),
                                OP.mult)
                        if g == 0:
                            oams[img] = scp.tile([P, 2, 2], F32, tag="sc",
                                                 name="oam")
                        for nt in range(2):
                            nc.vector.tensor_reduce(
                                oams[img][:, nt, g:g + 1],
                                o_tm[:, img * 2 + nt, 384 * g:384 * (g + 1)],
                                axis=AX.X, op=OP.max,
                                apply_absolute_value=True)
                        if g == 1:
                            o_quant_pair(img)

                    build_cb(0)
                    v_mm(0)
                    v_mm(1)
                    qk_mm(0)
                    build_cb(1)
                    v_mm(2)
                    v_mm(3)
                    qk_mm(1)
                    for img in range(2):
                        for g in range(2):
                            lt_grp(img, g)
                            oa_grp(img, g)
                    n2_ssq = [None, None]
                    for t in range(NT):
                        proj_t(t)
                        if t % 2 == 0:
                            n2_ssq[t // 2] = scp.tile([P, 2], F32, tag="sc",
                                                      name="ssq2")
                        ssq_act(z[:, t, :], n2_ssq[t // 2][:, t % 2:t % 2 + 1])
                        if t % 2 == 1:
                            n2_pair(t // 2)

                # --- fc1 + gelu + g-quant ---
                wf2_tiles, wf2_span = load_w(wfc2_d, b, KH, DIM, 3)
                xqg = [None] * NT
                c4s = [None] * NT
                gs = [None] * NT

                def gquant(t):
                    gh0, gh1 = gs[t]
                    am = scp.tile([P, 2], F32, tag="sc", name="amg")
                    nc.vector.tensor_reduce(am[:, 0:1], gh0[:], axis=AX.X, op=OP.max,
                                            apply_absolute_value=True)
                    nc.vector.tensor_reduce(am[:, 1:2], gh1[:], axis=AX.X, op=OP.max,
                                            apply_absolute_value=True)
                    ac = scp.tile([P, 1], F32, tag="sc", name="amaxc")
                    nc.vector.tensor_tensor(ac[:], am[:, 0:1], am[:, 1:2], OP.max)
                    rs = scp.tile([P, 1], F32, tag="sc", name="rcp")
                    nc.vector.reciprocal(rs[:], ac[:])
                    s127 = scp.tile([P, 1], F32, tag="sc", name="s127")
                    nc.vector.tensor_scalar_mul(s127[:], rs[:], 127.0)
                    c = scp.tile([P, 1], F32, tag="sc", name="cc")
                    nc.vector.tensor_scalar(c[:], ac[:], wsb[:, 4 * b + 3:4 * b + 4],
                                            None, OP.mult)
                    c4s[t] = c
                    xg = xqgp.tile([P, KH, P], BF16, tag="xqg")
                    xqg[t] = xg
                    for i, gh in enumerate((gh0, gh1)):
                        # magic on ACT (Identity, no table switch), unmagic DVE
                        nc.scalar.activation(gh[:], gh[:], AF.Identity,
                                             scale=s127[:], bias=pmag[:])
                        xq = xqsp.tile([P, HID // 2], BF16, tag="xqs", name="xq24s")
                        nc.vector.tensor_scalar(xq[:], gh[:], MAGIC, None, OP.subtract)
                        nc.sync.dma_start_transpose(xg[:, i * 12:(i + 1) * 12, :], xq[:])

                # --- fc1/fc2 interleaved per tile, fc2 fused with next p1 ---
                fuse = b + 1 < depth
                if fuse:
                    mt1_nxt = load_mods(b + 1, 0, "mt1")
                    xqT_cur = [xqtp.tile([P, KD, NTOK], BF16, tag="xqt",
                                          name=f"xqt{i}") for i in range(2)]
                    c_cur = [None] * NT

                def fc1_t(t):
                    gh0 = gp.tile([P, HID // 2], F32, tag="g")
                    gh1 = gp.tile([P, HID // 2], F32, tag="g")
                    gs[t] = (gh0, gh1)
                    for ci, (cs, ce) in enumerate(_mm_chunks(HID)):
                        pt = ps_mm.tile([P, 512], F32, tag="mm", name="pmm")[:, : ce - cs]
                        for k in range(KD):
                            wt = wf1_tiles[k // wf1_span]
                            nc.tensor.matmul(
                                pt[:], xq2T_h[t // 2][:, k, (t % 2) * P:(t % 2 + 1) * P],
                                wt[:, k % wf1_span, cs:ce],
                                start=(k == 0), stop=(k == KD - 1))
                        gh = gh0 if ci < 3 else gh1
                        off = cs - (0 if ci < 3 else HID // 2)
                        nc.scalar.activation(gh[:, off:off + ce - cs], pt[:],
                                             AF.Gelu_apprx_tanh, scale=c3s[t][:])

                def fc2_t(t, ssq_split=None):
                    for ci, (cs, ce) in enumerate(_mm_chunks(DIM)):
                        pt = ps_mm.tile([P, 512], F32, tag="mm", name="pmm")[:, : ce - cs]
                        for k in range(KH):
                            wt = wf2_tiles[k // wf2_span]
                            nc.tensor.matmul(pt[:], xqg[t][:, k, :],
                                             wt[:, k % wf2_span, cs:ce],
                                             start=(k == 0), stop=(k == KH - 1))
                        nc.vector.scalar_tensor_tensor(
                            z[:, t, cs:ce], pt[:], c4s[t][:], z[:, t, cs:ce],
                            OP.mult, OP.add)
                        if ssq_split is not None:
                            nc.scalar.activation(
                                sq_scr[:, cs:ce], z[:, t, cs:ce], AF.Square,
                                accum_out=ssq_split[:, ci:ci + 1])

                with nc.named_scope(f"b{b}_mlp"):
                    for t in range(NT):
                        fc1_t(t)
                        if t > 0:
                            gquant(t - 1)
                            if t == 3 and fuse:
                                ssq_t2 = scp.tile([P, 2], F32, tag="sc",
                                                  name="sspl2")
                                fc2_t(2, ssq_split=ssq_t2)
                            else:
                                fc2_t(t - 1)
                        if t == 2 and fuse:
                            p1_pair(0, mt1_nxt, 4 * (b + 1), xqT_cur[0], c_cur)
                        if t == 3 and fuse:
                            wq_next = load_w(wqkv_d, b + 1, KD, 3 * DIM, 3)
                    gquant(NT - 1)
                    if fuse:
                        p1_tile(2, mt1_nxt, 4 * (b + 1), xqT_cur[1], c_cur,
                                ssq_split=ssq_t2)
                        ssq_t3 = scp.tile([P, 2], F32, tag="sc", name="sspl3")
                        fc2_t(3, ssq_split=ssq_t3)
                        p1_tile(3, mt1_nxt, 4 * (b + 1), xqT_cur[1], c_cur,
                                ssq_split=ssq_t3)
                    else:
                        fc2_t(NT - 1)

            # ---------------- final norm + head ----------------
            with nc.named_scope("head"):
                hw_pieces = []
                for i in range(3):
                    hwp = wp.tile([P, 2, DIM], F32, tag="w", name="hwp")
                    nc.gpsimd.dma_start(
                        hwp[:], headWT_d[i * 2 * P:(i + 1) * 2 * P, :].rearrange(
                            "(o p) d -> p o d", p=P))
                    hw_pieces.append(hwp)
                hbrow = tmp_.tile([1, DIM], F32, tag="tm", name="hbrow")
                nc.sync.dma_start(hbrow[:], headb_d[:])
                hbb = wp.tile([P, DIM], F32, tag="w", name="hbb")
                nc.gpsimd.partition_broadcast(hbb[:], hbrow[0:1, :])
                rst_cols = []
                for pi in range(2):
                    ssq2 = scp.tile([P, 2], F32, tag="sc", name="ssqh")
                    for j in range(2):
                        ssq_act(z[:, 2 * pi + j, :], ssq2[:, j:j + 1])
                    rst2 = rstd_pair(ssq2[:])
                    rst_cols += [rst2[:, 0:1], rst2[:, 1:2]]
                for t in range(NT):
                    zn = hp.tile([P, DIM], F32, tag="h")
                    nc.vector.tensor_scalar_mul(zn[:], z[:, t, :], rst_cols[t])
                    znT = hp.tile([P, DIM], F32, tag="h")
                    for g0 in range(0, KD, 4):
                        gn = min(4, KD - g0)
                        ptb = ps_lt.tile([P, 512], F32, tag="lt", name="ptb")[:, : gn * P]
                        for j in range(gn):
                            nc.tensor.transpose(ptb[:, j * P:(j + 1) * P],
                                                zn[:, (g0 + j) * P:(g0 + j + 1) * P], idf[:])
                        nc.vector.tensor_copy(znT[:, g0 * P:(g0 + gn) * P], ptb[:])
                    for (cs, ce) in _mm_chunks(DIM):
                        pt = ps_mm.tile([P, 512], F32, tag="mm", name="pmm")[:, : ce - cs]
                        for k in range(KD):
                            nc.tensor.matmul(pt[:], znT[:, k * P:(k + 1) * P],
                                             hw_pieces[k // 2][:, k % 2, cs:ce],
                                             start=(k == 0), stop=(k == KD - 1))
                        ot = tmp_.tile([P, DIM], F32, tag="tm", name="ot")[:, : ce - cs]
                        nc.vector.tensor_tensor(ot[:], pt[:], hbb[:, cs:ce], OP.add)
                        nc.sync.dma_start(out_d[t * P:(t + 1) * P, cs:ce], ot[:])

    nc.compile()
    return nc


# ---------------------------------------------------------------------------
# host-side numerics (numpy, fp32 — matches jax CPU within ~1e-7)

def _gelu_tanh(x):
    x = x.astype(np.float32)
    c = np.float32(math.sqrt(2.0 / math.pi))
    return np.float32(0.5) * x * (np.float32(1.0) +
                                  np.tanh(c * (x + np.float32(0.044715) * x * x * x)))


def _time_embedding(t, t_w1, t_b1, t_w2, t_b2):
    half = DIM // 2
    freqs = np.exp(-np.log(10000.0) * np.arange(half, dtype=np.float32) / (half - 1)).astype(np.float32)
    args = t[:, None].astype(np.float32) * freqs[None, :]
    emb = np.concatenate([np.sin(args), np.cos(args)], axis=-1).astype(np.float32)
    h = _gelu_tanh(emb @ t_w1.T + t_b1)
    return (h @ t_w2.T + t_b2).astype(np.float32)


def _quant_w(w):
    ws = np.float32(np.mean(np.abs(w), dtype=np.float64)) + np.float32(1e-5)
    wq = np.clip(np.round(w.astype(np.float32) / ws), -1.0, 1.0)
    return wq, ws


def _prepare(inputs):
    x = np.asarray(inputs["x"], np.float32)
    t = np.asarray(inputs["t"], np.float32)
    B = x.shape[0]
    n_cores = 8
    per = B // n_cores  # 2
    p = PATCH
    hh = IMG // p

    xp = x.reshape(B, CIN, hh, p, hh, p).transpose(0, 2, 4, 1, 3, 5).reshape(B, hh * hh, CIN * p * p)

    t_emb = _time_embedding(t, inputs["t_w1"], inputs["t_b1"], inputs["t_w2"], inputs["t_b2"])
    silu = (t_emb / (1.0 + np.exp(-t_emb))).astype(np.float32)

    depth = DEPTH
    mods = np.zeros((depth, 2, B, 2, DIM), np.float32)  # [blk, norm, img, A/B, D]
    wscl = np.zeros((4 * depth,), np.float32)
    wq_all, wp_all, wf1_all, wf2_all = [], [], [], []
    for b in range(depth):
        mod = silu @ np.asarray(inputs["blk_ada_w"][b], np.float32).T + np.asarray(
            inputs["blk_ada_b"][b], np.float32)
        sh1, sc1, sh2, sc2 = np.split(mod, 4, axis=-1)
        n1 = np.asarray(inputs["blk_norm1"][b], np.float32)
        n2 = np.asarray(inputs["blk_norm2"][b], np.float32)
        mods[b, 0, :, 0, :] = n1[None, :] * (1.0 + sc1)
        mods[b, 0, :, 1, :] = sh1
        mods[b, 1, :, 0, :] = n2[None, :] * (1.0 + sc2)
        mods[b, 1, :, 1, :] = sh2

        for j, (nm, lst) in enumerate([("blk_qkv", wq_all), ("blk_proj", wp_all),
                                       ("blk_fc1", wf1_all), ("blk_fc2", wf2_all)]):
            wq, ws = _quant_w(np.asarray(inputs[nm][b], np.float32))
            lst.append(np.ascontiguousarray(wq.T).astype(ml_dtypes.float8_e4m3))
            wscl[4 * b + j] = ws / np.float32(127.0)

    wqkv = np.stack(wq_all)
    wproj = np.stack(wp_all)
    wfc1 = np.stack(wf1_all)
    wfc2 = np.stack(wf2_all)

    posb = (np.asarray(inputs["pos_embed"][0], np.float32) +
            np.asarray(inputs["patch_b"], np.float32)[None, :]).astype(np.float32)
    patchWT = np.ascontiguousarray(np.asarray(inputs["patch_w"], np.float32).T)
    norm_w = np.asarray(inputs["norm_w"], np.float32)
    headWT = np.ascontiguousarray(np.asarray(inputs["head_w"], np.float32).T * norm_w[:, None])
    headb = np.asarray(inputs["head_b"], np.float32)[None, :]

    key = ("prog", depth)
    if key not in _CACHED:
        _CACHED[key] = build_program(depth)
    nc = _CACHED[key]

    in_maps = []
    for c in range(n_cores):
        imgs = slice(c * per, (c + 1) * per)
        xpT = np.ascontiguousarray(xp[imgs].reshape(per * hh * hh, CIN * p * p).T)
        in_maps.append(dict(
            xpT=xpT, posb=posb, patchWT=patchWT, headWT=headWT, headb=headb,
            wqkv=wqkv, wproj=wproj, wfc1=wfc1, wfc2=wfc2,
            mods=np.ascontiguousarray(
                np.broadcast_to(mods[:, :, None, imgs], (depth, 2, 128, per, 2, DIM))),
            wscl=wscl[None, :],
        ))

    return nc, in_maps


def _assemble(res, B=16, per=2):
    p = PATCH
    hh = IMG // p
    out = np.zeros((B, CIN, IMG, IMG), np.float32)
    for c in range(B // per):
        zo = res.results[c]["zout"]  # [512, 768]
        for i in range(per):
            zi = zo[i * 256:(i + 1) * 256]
            out[c * per + i] = zi.reshape(hh, hh, CIN, p, p).transpose(2, 0, 3, 1, 4).reshape(CIN, IMG, IMG)
    return out


def kernel(**inputs):
    nc, in_maps = _prepare(inputs)
    res = run_bass_kernel_spmd(nc, in_maps, list(range(len(in_maps))), trace=False)
    return _assemble(res)


# revision 53
# speedup vs baseline: 1.2688x; 1.0375x over previous
"""BitNet DiT on 8 Trainium2 NeuronCores — data-parallel over batch (2 images/core).

Host: patchify, time-embedding + adaLN modulation vectors, BitNet weight
quantization (ternary * per-tensor scale) -> fp8 upload.
Device: full 12-block DiT forward per core in a single Bass/Tile kernel.
BitNet matmuls run as exact integer arithmetic in bf16 (|values| <= 127,
fp32 accumulate). Attention runs via transposed-logits + ones-column
softmax-denominator trick.

v3 (vs v2):
- quant chains (ssq, rstd, amax, magic rounds) moved to DVE; rstd via
  Newton-Raphson rsqrt (bit trick + 2 iters) -> no Sqrt ACT table loads
  (only 2 table switches/block: exp <-> gelu).
- q/k computed weight-stationary producing [feat, tok] layout directly:
  kills 48 PE transposes + ACT evacs per block. Per-token activation
  scales applied via a broadcast c-row tile (PE column-transpose trick).
- attention softmax normalization batched: 6 heads per PSUM tile, one
  strided reciprocal + one broadcast multiply (was 48 ACT ops/block).
- softmax weights (eT) in bf16: halves SBUF + faster AV LDWEIGHTS.
- phase order tuned so PE never idles >3.4us (HAM stays at 2.4 GHz):
  v-matmuls before q/k, o-quant interleaved with attention tail.
"""
import math
import os
import sys
import numpy as np

sys.path.insert(0, "/opt/trn_rl_repo")

import ml_dtypes  # noqa: E402
import concourse.bass as bass  # noqa: E402
import concourse.mybir as mybir  # noqa: E402
import concourse.tile as tile  # noqa: E402
from concourse import bacc  # noqa: E402
from concourse.bass_utils import run_bass_kernel_spmd  # noqa: E402
from concourse.masks import make_identity  # noqa: E402

F32 = mybir.dt.float32
F32R = mybir.dt.float32r
I32 = mybir.dt.int32
FP8 = mybir.dt.float8e4
BF16 = mybir.dt.bfloat16
AX = mybir.AxisListType
OP = mybir.AluOpType
AF = mybir.ActivationFunctionType

DIM = 768
DEPTH = int(os.environ.get("KERNEL_DEPTH", "12"))
HEADS = 12
HD = 64
PATCH = 16
IMG = 256
CIN = 3
HID = 4 * DIM
EPS = 1e-6
P = 128
T = 512            # tokens per core (2 images x 256)
NT = T // P        # 4 token tiles
NTOK = 256         # tokens per image
KD = DIM // P      # 6
KH = HID // P      # 24
MAGIC = float(np.float32(3 * 2**22))  # 12582912.0 RNE round-to-int magic
RSQRT_SEED = 0x5F375A86

_CACHED = {}


def _mm_chunks(n):
    out = []
    s = 0
    while s < n:
        e = min(s + 512, n)
        out.append((s, e))
        s = e
    return out


def build_program(depth=DEPTH):
    nc = bacc.Bacc("TRN2", target_bir_lowering=False, debug=False, num_devices=8)

    xpT_d = nc.declare_dram_parameter("xpT", [DIM, T], F32, isOutput=False)
    posb_d = nc.declare_dram_parameter("posb", [NTOK, DIM], F32, isOutput=False)
    patchWT_d = nc.declare_dram_parameter("patchWT", [DIM, DIM], F32, isOutput=False)
    headWT_d = nc.declare_dram_parameter("headWT", [DIM, DIM], F32, isOutput=False)
    headb_d = nc.declare_dram_parameter("headb", [1, DIM], F32, isOutput=False)
    wqkv_d = nc.declare_dram_parameter("wqkv", [depth, DIM, 3 * DIM], FP8, isOutput=False)
    wproj_d = nc.declare_dram_parameter("wproj", [depth, DIM, DIM], FP8, isOutput=False)
    wfc1_d = nc.declare_dram_parameter("wfc1", [depth, DIM, HID], FP8, isOutput=False)
    wfc2_d = nc.declare_dram_parameter("wfc2", [depth, HID, DIM], FP8, isOutput=False)
    # modulation vectors: [block, norm(2), part, img(2), A/B(2), 768]
    mods_d = nc.declare_dram_parameter("mods", [depth, 2, P, 2, 2, DIM], F32, isOutput=False)
    wscl_d = nc.declare_dram_parameter("wscl", [1, 4 * depth], F32, isOutput=False)
    out_d = nc.declare_dram_parameter("zout", [T, DIM], F32, isOutput=True)

    with tile.TileContext(nc) as tc:
        from contextlib import ExitStack
        with ExitStack() as _ctx:
            constp = _ctx.enter_context(tc.tile_pool(name="const", bufs=1))
            residp = _ctx.enter_context(tc.tile_pool(name="resid", bufs=1))
            qkp = _ctx.enter_context(tc.tile_pool(name="qk", bufs=2))
            wp = _ctx.enter_context(tc.tile_pool(name="w", bufs=6))
            modp = _ctx.enter_context(tc.tile_pool(name="mod", bufs=2))
            tmp_ = _ctx.enter_context(tc.tile_pool(name="tm", bufs=2))
            gp = _ctx.enter_context(tc.tile_pool(name="g", bufs=3))
            hp = _ctx.enter_context(tc.tile_pool(name="h", bufs=4))
            xqtp = _ctx.enter_context(tc.tile_pool(name="xqt", bufs=4))
            xqop = _ctx.enter_context(tc.tile_pool(name="xqo", bufs=2))
            xq2p = _ctx.enter_context(tc.tile_pool(name="xq2", bufs=2))
            xqgp = _ctx.enter_context(tc.tile_pool(name="xqg", bufs=2))
            xqsp = _ctx.enter_context(tc.tile_pool(name="xqs", bufs=4))
            eTp = _ctx.enter_context(tc.tile_pool(name="eT", bufs=2))
            cbp = _ctx.enter_context(tc.tile_pool(name="cb", bufs=3))
            scp = _ctx.enter_context(tc.tile_pool(name="sc", bufs=64))
            ps_mm = _ctx.enter_context(tc.tile_pool(name="ps_mm", bufs=3, space="PSUM"))
            ps_lt = _ctx.enter_context(tc.tile_pool(name="ps_lt", bufs=2, space="PSUM"))
            ps_oa = _ctx.enter_context(tc.tile_pool(name="ps_oa", bufs=2, space="PSUM"))
            ps_x = _ctx.enter_context(tc.tile_pool(name="ps_x", bufs=1, space="PSUM"))

            idf = constp.tile([P, P], F32)
            make_identity(nc, idf[:])

            # broadcast w_scales/127 to all partitions
            wsrow = constp.tile([1, 4 * depth], F32)
            nc.sync.dma_start(wsrow[:], wscl_d[:])
            wsb = constp.tile([P, 4 * depth], F32)
            nc.gpsimd.partition_broadcast(wsb[:], wsrow[0:1, :])
            pmag = constp.tile([P, 1], F32)
            nc.vector.memset(pmag[:], MAGIC)

            z = residp.tile([P, NT, DIM], F32)
            v_aug = residp.tile([P, NT, HEADS, HD + 1], BF16)
            nc.vector.memset(v_aug[:, :, :, HD], 1.0)
            o_tm = residp.tile([P, NT, DIM], F32)

            # ---------- quant-chain helpers ----------
            # rsqrt seed for doubled input: rsqrt(2*hx) where hx = ms/2
            K2 = RSQRT_SEED - 0x400000

            def rstd_pair(ssq2):
                """[P,n] rstd = 1/sqrt(ssq/DIM+EPS), bit-trick + 2 Newton iters.
                All DVE; batched over the input's columns."""
                n = ssq2.shape[-1]
                hx = scp.tile([P, n], F32, tag="sc", name="hx")
                nc.vector.tensor_scalar(hx[:], ssq2, 0.5 / DIM, EPS / 2,
                                        OP.mult, OP.add)
                yi = scp.tile([P, n], I32, tag="sc", name="yi")
                nc.vector.tensor_scalar(yi[:], hx[:].bitcast(I32), 1, None,
                                        OP.arith_shift_right)
                yn = scp.tile([P, n], I32, tag="sc", name="yn")
                nc.vector.tensor_scalar(yn[:], yi[:], -1, None, OP.bitwise_xor)
                y = scp.tile([P, n], I32, tag="sc", name="y0")
                nc.vector.tensor_scalar(y[:], yn[:], K2 + 1, None, OP.add)
                yf = y[:].bitcast(F32)
                for it in range(1):
                    a = scp.tile([P, n], F32, tag="sc", name=f"nra{it}")
                    nc.vector.tensor_tensor(a[:], yf, yf, OP.mult)
                    b = scp.tile([P, n], F32, tag="sc", name=f"nrb{it}")
                    nc.vector.tensor_tensor(b[:], a[:], hx[:], OP.mult)
                    c = scp.tile([P, n], F32, tag="sc", name=f"nrc{it}")
                    nc.vector.tensor_scalar(c[:], b[:], -1.0, 1.5, OP.mult, OP.add)
                    y2 = scp.tile([P, n], F32, tag="sc", name=f"nry{it}")
                    nc.vector.tensor_tensor(y2[:], yf, c[:], OP.mult)
                    yf = y2[:]
                return yf

            sq_scr = tmp_.tile([P, DIM], F32, tag="tm", name="sqscr")

            def ssq_act(src_ap, sv_col):
                """sum(src^2) per row on ACT (Square table-free), accum into
                the given [P,1] column."""
                nc.scalar.activation(sq_scr[:], src_ap, AF.Square,
                                     accum_out=sv_col)

            def norm_mod(t, mt, rstd_ap, dst):
                """dst = (z[t]*rstd) * modA + modB  (2 DVE ops)."""
                img = t // 2
                nc.vector.scalar_tensor_tensor(dst, z[:, t, :], rstd_ap,
                                               mt[:, img, 0, :], OP.mult, OP.mult)
                nc.vector.tensor_tensor(dst, dst, mt[:, img, 1, :], OP.add)

            def quant_scales(amax, n, ws_idx):
                """(s127[P,n], c[P,n]) from a filled amax tile."""
                rs = scp.tile([P, n], F32, tag="sc", name="rcp")
                nc.vector.reciprocal(rs[:], amax[:, 0:n])
                s127 = scp.tile([P, n], F32, tag="sc", name="s127")
                nc.vector.tensor_scalar_mul(s127[:], rs[:], 127.0)
                c = scp.tile([P, n], F32, tag="sc", name="cc")
                nc.vector.tensor_scalar(c[:], amax[:, 0:n],
                                        wsb[:, ws_idx:ws_idx + 1], None, OP.mult)
                return s127, c

            def quant_pair(src_aps, ws_idx):
                """amax over each source -> batched (s127[P,n], c[P,n])."""
                n = len(src_aps)
                amax = scp.tile([P, n], F32, tag="sc", name="amax")
                for j, src in enumerate(src_aps):
                    nc.vector.tensor_reduce(amax[:, j:j + 1], src, axis=AX.X,
                                            op=OP.max, apply_absolute_value=True)
                return quant_scales(amax, n, ws_idx)

            def round_dma_act(src_ap, s127_col, dst_slice):
                """magic-round on ACT (in-place), unmagic to bf16 on DVE,
                DMA-transpose into dst."""
                nc.scalar.activation(src_ap, src_ap, AF.Identity,
                                     scale=s127_col, bias=pmag[:])
                xq = xqsp.tile([P, DIM], BF16, tag="xqs", name="xqs")
                nc.vector.tensor_scalar(xq[:], src_ap, MAGIC, None, OP.subtract)
                nc.sync.dma_start_transpose(dst_slice, xq[:])

            def round_dma_dve(src_ap, s127_col, dst_slice):
                """magic-round + unmagic both on DVE (for attn phase where
                ACT is busy with exp)."""
                nc.vector.tensor_scalar(src_ap, src_ap, s127_col, MAGIC,
                                        OP.mult, OP.add)
                xq = xqsp.tile([P, DIM], BF16, tag="xqs", name="xqs")
                nc.vector.tensor_scalar(xq[:], src_ap, MAGIC, None, OP.subtract)
                nc.sync.dma_start_transpose(dst_slice, xq[:])

            # ---------------- patch embed ----------------
            posb_sb = wp.tile([P, 2, DIM], F32, tag="w", name="posb_sb")
            nc.sync.dma_start(posb_sb[:], posb_d.rearrange("(a p) d -> p a d", p=P))
            xpT_h = []
            for hf in range(2):
                xh = qkp.tile([P, KD, NTOK], F32, tag="qk", name="xpT")
                nc.sync.dma_start(
                    xh[:], xpT_d[:, hf * NTOK:(hf + 1) * NTOK].rearrange(
                        "(o p) t -> p o t", p=P))
                xpT_h.append(xh)
            pw_pieces = []
            for i in range(3):
                pwp = wp.tile([P, 2, DIM], F32, tag="w", name="pwp")
                nc.gpsimd.dma_start(
                    pwp[:], patchWT_d[i * 2 * P:(i + 1) * 2 * P, :].rearrange(
                        "(o p) d -> p o d", p=P))
                pw_pieces.append(pwp)
            for t in range(NT):
                for (cs, ce) in _mm_chunks(DIM):
                    pt = ps_mm.tile([P, 512], F32, tag="mm", name="pmm")[:, : ce - cs]
                    for k in range(KD):
                        nc.tensor.matmul(pt[:], xpT_h[t // 2][:, k, (t % 2) * P:(t % 2 + 1) * P],
                                         pw_pieces[k // 2][:, k % 2, cs:ce],
                                         start=(k == 0), stop=(k == KD - 1))
                    nc.vector.tensor_tensor(z[:, t, cs:ce], pt[:], posb_sb[:, t % 2, cs:ce], OP.add)

            def load_w(dram, b, kchunks, width, npieces):
                """Stage one linear's transposed fp8 weights as npieces tiles."""
                span = kchunks // npieces
                tiles = []
                for i in range(npieces):
                    wt = wp.tile([P, span, width], FP8, tag="w")
                    nc.gpsimd.dma_start(
                        wt[:],
                        dram[b, i * span * P:(i + 1) * span * P, :].rearrange(
                            "(o p) f -> p o f", p=P))
                    tiles.append(wt)
                return tiles, span

            def load_mods(b_, n_, name):
                mt = modp.tile([P, 2, 2, DIM], F32, tag="mod", name=name)
                nc.gpsimd.dma_start(mt[:], mods_d[b_, n_])
                return mt

            def p1_pair(pi, mt, ws_idx, half_dst, c_out, ssq2=None):
                """norm1/norm2 + quant for tile pair (2pi, 2pi+1); writes the
                transposed quantized half tile [P, KD, NTOK]."""
                if ssq2 is None:
                    ssq2 = scp.tile([P, 2], F32, tag="sc", name="ssq2")
                    for j in range(2):
                        ssq_act(z[:, 2 * pi + j, :], ssq2[:, j:j + 1])
                rst2 = rstd_pair(ssq2[:])
                hs = []
                for j in range(2):
                    h = hp.tile([P, DIM], F32, tag="h")
                    norm_mod(2 * pi + j, mt, rst2[:, j:j + 1], h[:])
                    hs.append(h)
                s127, cpr = quant_pair([h[:] for h in hs], ws_idx)
                for j in range(2):
                    c_out[2 * pi + j] = cpr[:, j:j + 1]
                    round_dma_act(hs[j][:], s127[:, j:j + 1],
                                  half_dst[:, :, j * P:(j + 1) * P])

            def p1_tile(t, mt, ws_idx, half_dst, c_out):
                """single-tile norm+quant (for latency-critical tail tiles)."""
                ssq1 = scp.tile([P, 1], F32, tag="sc", name="ssq1")
                ssq_act(z[:, t, :], ssq1[:])
                rst = rstd_pair(ssq1[:])
                h = hp.tile([P, DIM], F32, tag="h")
                norm_mod(t, mt, rst, h[:])
                s127, cpr = quant_pair([h[:]], ws_idx)
                c_out[t] = cpr[:, 0:1]
                round_dma_act(h[:], s127[:, 0:1],
                              half_dst[:, :, (t % 2) * P:(t % 2 + 1) * P])

            # ---- prologue: phase 1 of block 0 ----
            mt1_nxt = load_mods(0, 0, "mt1")
            xqT_cur = [xqtp.tile([P, KD, NTOK], BF16, tag="xqt", name="xqt0"), xqtp.tile([P, KD, NTOK], BF16, tag="xqt", name="xqt1")]
            c_cur = [None] * NT
            for pi in range(2):
                p1_pair(pi, mt1_nxt, 0, xqT_cur[pi], c_cur)

            wq_next = load_w(wqkv_d, 0, KD, 3 * DIM, 3)
            for b in range(depth):
                xqT_h, c_list = xqT_cur, c_cur
                mt2 = load_mods(b, 1, "mt2")
                wq_tiles, wq_span = wq_next

                with nc.named_scope(f"b{b}_qkv"):
                    q_fm = qkp.tile([P, KD, T], BF16, tag="qk", name="q_fm")
                    k_fm = qkp.tile([P, KD, T], BF16, tag="qk", name="k_fm")
                    cb = cbp.tile([P, T], F32, tag="cb", name="cbb")

                    def build_cb(half):
                        # per-token scales -> [1,256] row via PE transpose,
                        # then broadcast to all partitions.
                        crow_ps = ps_x.tile([1, 2, P], F32, tag="x", name="crow_ps")
                        for j in range(2):
                            nc.tensor.transpose(crow_ps[:, j, :],
                                                c_list[2 * half + j], idf[:])
                        crow = cbp.tile([1, NTOK], F32, tag="cb", name="crow")
                        nc.vector.tensor_copy(
                            crow[:], crow_ps[:].rearrange("a b c -> a (b c)"))
                        nc.gpsimd.partition_broadcast(
                            cb[:, half * NTOK:(half + 1) * NTOK], crow[0:1, :])

                    def v_mm(t):
                        for (cs, ce) in _mm_chunks(DIM):
                            pt = ps_mm.tile([P, 512], F32, tag="mm", name="pmm")[:, : ce - cs]
                            for k in range(KD):
                                wt = wq_tiles[k // wq_span]
                                nc.tensor.matmul(
                                    pt[:], xqT_h[t // 2][:, k, (t % 2) * P:(t % 2 + 1) * P],
                                    wt[:, k % wq_span, 2 * DIM + cs:2 * DIM + ce],
                                    start=(k == 0), stop=(k == KD - 1))
                            nc.scalar.activation(
                                v_aug[:, t, cs // HD:ce // HD, 0:HD], pt[:],
                                AF.Identity, scale=c_list[t])

                    def qk_mm(half):
                        # weight-stationary, psum [feat 128, tok 256]
                        ts0 = half * NTOK
                        for fc in range(12):
                            pt = ps_mm.tile([P, 512], F32, tag="mm",
                                            name="pmm")[:, :NTOK]
                            for k in range(KD):
                                wt = wq_tiles[k // wq_span]
                                nc.tensor.matmul(
                                    pt[:], wt[:, k % wq_span, fc * P:(fc + 1) * P],
                                    xqT_h[half][:, k, :],
                                    start=(k == 0), stop=(k == KD - 1))
                            if fc < 6:
                                nc.vector.scalar_tensor_tensor(
                                    q_fm[:, fc, ts0:ts0 + NTOK], pt[:], 0.125,
                                    cb[:, ts0:ts0 + NTOK], OP.mult, OP.mult)
                            else:
                                nc.vector.tensor_tensor(
                                    k_fm[:, fc - 6, ts0:ts0 + NTOK], pt[:],
                                    cb[:, ts0:ts0 + NTOK], OP.mult)

                    # proj weights + o-quant dst staged up front
                    wp_tiles, wp_span = load_w(wproj_d, b, KD, DIM, 2)
                    xqoT_h = [xqop.tile([P, KD, NTOK], BF16, tag="xqo",
                                        name=f"xqo{i}") for i in range(2)]
                    co_s = [None] * NT

                    def o_quant_pair(img):
                        srcs = [o_tm[:, 2 * img + j, :] for j in range(2)]
                        s127, cpr = quant_pair(srcs, 4 * b + 1)
                        for j in range(2):
                            co_s[2 * img + j] = cpr[:, j:j + 1]
                            round_dma_dve(srcs[j], s127[:, j:j + 1],
                                          xqoT_h[img][:, :, j * P:(j + 1) * P])

                    wf1_tiles, wf1_span = load_w(wfc1_d, b, KD, HID, 3)
                    xq2T_h = [xq2p.tile([P, KD, NTOK], BF16, tag="xq2",
                                        name=f"xq2{i}") for i in range(2)]
                    c3s = [None] * NT

                    def proj_t(t):
                        for (cs, ce) in _mm_chunks(DIM):
                            pt = ps_mm.tile([P, 512], F32, tag="mm", name="pmm")[:, : ce - cs]
                            for k in range(KD):
                                wt = wp_tiles[k // wp_span]
                                nc.tensor.matmul(
                                    pt[:], xqoT_h[t // 2][:, k, (t % 2) * P:(t % 2 + 1) * P],
                                    wt[:, k % wp_span, cs:ce],
                                    start=(k == 0), stop=(k == KD - 1))
                            nc.vector.scalar_tensor_tensor(
                                z[:, t, cs:ce], pt[:], co_s[t], z[:, t, cs:ce],
                                OP.mult, OP.add)

                    def n2_pair(pi):
                        p1_pair(pi, mt2, 4 * b + 2, xq2T_h[pi], c3s,
                                ssq2=n2_ssq[pi])

                    eTgs = {}

                    def lt_grp(img, g):
                        eTg = eTp.tile([P, 6, 2, NTOK], BF16, tag="eT")
                        eTgs[(img, g)] = eTg
                        for h6 in range(6):
                            hh = 6 * g + h6
                            po = (hh % 2) * HD
                            ch = hh // 2
                            lt = ps_lt.tile([P, 2, NTOK], F32, tag="lt")
                            for mt in range(2):
                                nc.tensor.matmul(
                                    lt[:, mt, :],
                                    k_fm[po:po + HD, ch,
                                         img * NTOK + mt * P: img * NTOK + (mt + 1) * P],
                                    q_fm[po:po + HD, ch,
                                         img * NTOK: (img + 1) * NTOK],
                                    start=True, stop=True)
                            nc.scalar.activation(eTg[:, h6], lt[:], AF.Exp)

                    def oa_grp(img, g):
                        eTg = eTgs.pop((img, g))
                        for nt in range(2):
                            oa = ps_oa.tile([P, 6, HD + 1], F32, tag="oa")
                            for h6 in range(6):
                                for mt in range(2):
                                    nc.tensor.matmul(
                                        oa[:, h6, :],
                                        eTg[:, h6, mt, nt * P:(nt + 1) * P],
                                        v_aug[:, img * 2 + mt, 6 * g + h6, :],
                                        start=(mt == 0), stop=(mt == 1))
                            rinv = scp.tile([P, 6], F32, tag="sc", name="rinv")
                            nc.vector.reciprocal(rinv[:], oa[:, :, HD])
                            dst = o_tm[:, img * 2 + nt,
                                       384 * g:384 * (g + 1)].rearrange(
                                           "p (h d) -> p h d", h=6)
                            nc.vector.tensor_tensor(
                                dst, oa[:, :, 0:HD],
                                rinv[:, :, None].broadcast_to([P, 6, HD]),
                                OP.mult)
                        if g == 1:
                            o_quant_pair(img)

                    build_cb(0)
                    v_mm(0)
                    v_mm(1)
                    qk_mm(0)
                    build_cb(1)
                    v_mm(2)
                    v_mm(3)
                    qk_mm(1)
                    for img in range(2):
                        for g in range(2):
                            lt_grp(img, g)
                            oa_grp(img, g)
                    n2_ssq = [None, None]
                    for t in range(NT):
                        proj_t(t)
                        if t % 2 == 0:
                            n2_ssq[t // 2] = scp.tile([P, 2], F32, tag="sc",
                                                      name="ssq2")
                        ssq_act(z[:, t, :], n2_ssq[t // 2][:, t % 2:t % 2 + 1])
                        if t % 2 == 1:
                            n2_pair(t // 2)

                # --- fc1 + gelu + g-quant ---
                wf2_tiles, wf2_span = load_w(wfc2_d, b, KH, DIM, 3)
                xqg = [None] * NT
                c4s = [None] * NT
                gs = [None] * NT

                def gquant(t):
                    gh0, gh1 = gs[t]
                    am = scp.tile([P, 2], F32, tag="sc", name="amg")
                    nc.vector.tensor_reduce(am[:, 0:1], gh0[:], axis=AX.X, op=OP.max,
                                            apply_absolute_value=True)
                    nc.vector.tensor_reduce(am[:, 1:2], gh1[:], axis=AX.X, op=OP.max,
                                            apply_absolute_value=True)
                    ac = scp.tile([P, 1], F32, tag="sc", name="amaxc")
                    nc.vector.tensor_tensor(ac[:], am[:, 0:1], am[:, 1:2], OP.max)
                    rs = scp.tile([P, 1], F32, tag="sc", name="rcp")
                    nc.vector.reciprocal(rs[:], ac[:])
                    s127 = scp.tile([P, 1], F32, tag="sc", name="s127")
                    nc.vector.tensor_scalar_mul(s127[:], rs[:], 127.0)
                    c = scp.tile([P, 1], F32, tag="sc", name="cc")
                    nc.vector.tensor_scalar(c[:], ac[:], wsb[:, 4 * b + 3:4 * b + 4],
                                            None, OP.mult)
                    c4s[t] = c
                    xg = xqgp.tile([P, KH, P], BF16, tag="xqg")
                    xqg[t] = xg
                    for i, gh in enumerate((gh0, gh1)):
                        # magic on ACT (Identity, no table switch), unmagic DVE
                        nc.scalar.activation(gh[:], gh[:], AF.Identity,
                                             scale=s127[:], bias=pmag[:])
                        xq = xqsp.tile([P, HID // 2], BF16, tag="xqs", name="xq24s")
                        nc.vector.tensor_scalar(xq[:], gh[:], MAGIC, None, OP.subtract)
                        nc.sync.dma_start_transpose(xg[:, i * 12:(i + 1) * 12, :], xq[:])

                # --- fc1/fc2 interleaved per tile, fc2 fused with next p1 ---
                fuse = b + 1 < depth
                if fuse:
                    mt1_nxt = load_mods(b + 1, 0, "mt1")
                    xqT_cur = [xqtp.tile([P, KD, NTOK], BF16, tag="xqt",
                                          name=f"xqt{i}") for i in range(2)]
                    c_cur = [None] * NT

                def fc1_t(t):
                    gh0 = gp.tile([P, HID // 2], F32, tag="g")
                    gh1 = gp.tile([P, HID // 2], F32, tag="g")
                    gs[t] = (gh0, gh1)
                    for ci, (cs, ce) in enumerate(_mm_chunks(HID)):
                        pt = ps_mm.tile([P, 512], F32, tag="mm", name="pmm")[:, : ce - cs]
                        for k in range(KD):
                            wt = wf1_tiles[k // wf1_span]
                            nc.tensor.matmul(
                                pt[:], xq2T_h[t // 2][:, k, (t % 2) * P:(t % 2 + 1) * P],
                                wt[:, k % wf1_span, cs:ce],
                                start=(k == 0), stop=(k == KD - 1))
                        gh = gh0 if ci < 3 else gh1
                        off = cs - (0 if ci < 3 else HID // 2)
                        nc.scalar.activation(gh[:, off:off + ce - cs], pt[:],
                                             AF.Gelu_apprx_tanh, scale=c3s[t][:])

                def fc2_t(t):
                    for (cs, ce) in _mm_chunks(DIM):
                        pt = ps_mm.tile([P, 512], F32, tag="mm", name="pmm")[:, : ce - cs]
                        for k in range(KH):
                            wt = wf2_tiles[k // wf2_span]
                            nc.tensor.matmul(pt[:], xqg[t][:, k, :],
                                             wt[:, k % wf2_span, cs:ce],
                                             start=(k == 0), stop=(k == KH - 1))
                        nc.vector.scalar_tensor_tensor(
                            z[:, t, cs:ce], pt[:], c4s[t][:], z[:, t, cs:ce],
                            OP.mult, OP.add)

                with nc.named_scope(f"b{b}_mlp"):
                    for t in range(NT):
                        fc1_t(t)
                        if t > 0:
                            gquant(t - 1)
                            fc2_t(t - 1)
                        if t == 2 and fuse:
                            p1_pair(0, mt1_nxt, 4 * (b + 1), xqT_cur[0], c_cur)
                        if t == 3 and fuse:
                            wq_next = load_w(wqkv_d, b + 1, KD, 3 * DIM, 3)
                    gquant(NT - 1)
                    if fuse:
                        p1_tile(2, mt1_nxt, 4 * (b + 1), xqT_cur[1], c_cur)
                    fc2_t(NT - 1)
                    if fuse:
                        p1_tile(3, mt1_nxt, 4 * (b + 1), xqT_cur[1], c_cur)

            # ---------------- final norm + head ----------------
            with nc.named_scope("head"):
                hw_pieces = []
                for i in range(3):
                    hwp = wp.tile([P, 2, DIM], F32, tag="w", name="hwp")
                    nc.gpsimd.dma_start(
                        hwp[:], headWT_d[i * 2 * P:(i + 1) * 2 * P, :].rearrange(
                            "(o p) d -> p o d", p=P))
                    hw_pieces.append(hwp)
                hbrow = tmp_.tile([1, DIM], F32, tag="tm", name="hbrow")
                nc.sync.dma_start(hbrow[:], headb_d[:])
                hbb = wp.tile([P, DIM], F32, tag="w", name="hbb")
                nc.gpsimd.partition_broadcast(hbb[:], hbrow[0:1, :])
                rst_cols = []
                for pi in range(2):
                    ssq2 = scp.tile([P, 2], F32, tag="sc", name="ssqh")
                    for j in range(2):
                        ssq_act(z[:, 2 * pi + j, :], ssq2[:, j:j + 1])
                    rst2 = rstd_pair(ssq2[:])
                    rst_cols += [rst2[:, 0:1], rst2[:, 1:2]]
                for t in range(NT):
                    zn = hp.tile([P, DIM], F32, tag="h")
                    nc.vector.tensor_scalar_mul(zn[:], z[:, t, :], rst_cols[t])
                    znT = hp.tile([P, DIM], F32, tag="h")
                    for g0 in range(0, KD, 4):
                        gn = min(4, KD - g0)
                        ptb = ps_lt.tile([P, 512], F32, tag="lt", name="ptb")[:, : gn * P]
                        for j in range(gn):
                            nc.tensor.transpose(ptb[:, j * P:(j + 1) * P],
                                                zn[:, (g0 + j) * P:(g0 + j + 1) * P], idf[:])
                        nc.vector.tensor_copy(znT[:, g0 * P:(g0 + gn) * P], ptb[:])
                    for (cs, ce) in _mm_chunks(DIM):
                        pt = ps_mm.tile([P, 512], F32, tag="mm", name="pmm")[:, : ce - cs]
                        for k in range(KD):
                            nc.tensor.matmul(pt[:], znT[:, k * P:(k + 1) * P],
                                             hw_pieces[k // 2][:, k % 2, cs:ce],
                                             start=(k == 0), stop=(k == KD - 1))
                        ot = tmp_.tile([P, DIM], F32, tag="tm", name="ot")[:, : ce - cs]
                        nc.vector.tensor_tensor(ot[:], pt[:], hbb[:, cs:ce], OP.add)
                        nc.sync.dma_start(out_d[t * P:(t + 1) * P, cs:ce], ot[:])

    nc.compile()
    return nc


# ---------------------------------------------------------------------------
# host-side numerics (numpy, fp32 — matches jax CPU within ~1e-7)

def _gelu_tanh(x):
    x = x.astype(np.float32)
    c = np.float32(math.sqrt(2.0 / math.pi))
    return np.float32(0.5) * x * (np.float32(1.0) +
                                  np.tanh(c * (x + np.float32(0.044715) * x * x * x)))


def _time_embedding(t, t_w1, t_b1, t_w2, t_b2):
    half = DIM // 2
    freqs = np.exp(-np.log(10000.0) * np.arange(half, dtype=np.float32) / (half - 1)).astype(np.float32)
    args = t[:, None].astype(np.float32) * freqs[None, :]
    emb = np.concatenate([np.sin(args), np.cos(args)], axis=-1).astype(np.float32)
    h = _gelu_tanh(emb @ t_w1.T + t_b1)
    return (h @ t_w2.T + t_b2).astype(np.float32)


def _quant_w(w):
    ws = np.float32(np.mean(np.abs(w), dtype=np.float64)) + np.float32(1e-5)
    wq = np.clip(np.round(w.astype(np.float32) / ws), -1.0, 1.0)
    return wq, ws


def _prepare(inputs):
    x = np.asarray(inputs["x"], np.float32)
    t = np.asarray(inputs["t"], np.float32)
    B = x.shape[0]
    n_cores = 8
    per = B // n_cores  # 2
    p = PATCH
    hh = IMG // p

    xp = x.reshape(B, CIN, hh, p, hh, p).transpose(0, 2, 4, 1, 3, 5).reshape(B, hh * hh, CIN * p * p)

    t_emb = _time_embedding(t, inputs["t_w1"], inputs["t_b1"], inputs["t_w2"], inputs["t_b2"])
    silu = (t_emb / (1.0 + np.exp(-t_emb))).astype(np.float32)

    depth = DEPTH
    mods = np.zeros((depth, 2, B, 2, DIM), np.float32)  # [blk, norm, img, A/B, D]
    wscl = np.zeros((4 * depth,), np.float32)
    wq_all, wp_all, wf1_all, wf2_all = [], [], [], []
    for b in range(depth):
        mod = silu @ np.asarray(inputs["blk_ada_w"][b], np.float32).T + np.asarray(
            inputs["blk_ada_b"][b], np.float32)
        sh1, sc1, sh2, sc2 = np.split(mod, 4, axis=-1)
        n1 = np.asarray(inputs["blk_norm1"][b], np.float32)
        n2 = np.asarray(inputs["blk_norm2"][b], np.float32)
        mods[b, 0, :, 0, :] = n1[None, :] * (1.0 + sc1)
        mods[b, 0, :, 1, :] = sh1
        mods[b, 1, :, 0, :] = n2[None, :] * (1.0 + sc2)
        mods[b, 1, :, 1, :] = sh2

        for j, (nm, lst) in enumerate([("blk_qkv", wq_all), ("blk_proj", wp_all),
                                       ("blk_fc1", wf1_all), ("blk_fc2", wf2_all)]):
            wq, ws = _quant_w(np.asarray(inputs[nm][b], np.float32))
            lst.append(np.ascontiguousarray(wq.T).astype(ml_dtypes.float8_e4m3))
            wscl[4 * b + j] = ws / np.float32(127.0)

    wqkv = np.stack(wq_all)
    wproj = np.stack(wp_all)
    wfc1 = np.stack(wf1_all)
    wfc2 = np.stack(wf2_all)

    posb = (np.asarray(inputs["pos_embed"][0], np.float32) +
            np.asarray(inputs["patch_b"], np.float32)[None, :]).astype(np.float32)
    patchWT = np.ascontiguousarray(np.asarray(inputs["patch_w"], np.float32).T)
    norm_w = np.asarray(inputs["norm_w"], np.float32)
    headWT = np.ascontiguousarray(np.asarray(inputs["head_w"], np.float32).T * norm_w[:, None])
    headb = np.asarray(inputs["head_b"], np.float32)[None, :]

    key = ("prog", depth)
    if key not in _CACHED:
        _CACHED[key] = build_program(depth)
    nc = _CACHED[key]

    in_maps = []
    for c in range(n_cores):
        imgs = slice(c * per, (c + 1) * per)
        xpT = np.ascontiguousarray(xp[imgs].reshape(per * hh * hh, CIN * p * p).T)
        in_maps.append(dict(
            xpT=xpT, posb=posb, patchWT=patchWT, headWT=headWT, headb=headb,
            wqkv=wqkv, wproj=wproj, wfc1=wfc1, wfc2=wfc2,
            mods=np.ascontiguousarray(
                np.broadcast_to(mods[:, :, None, imgs], (depth, 2, 128, per, 2, DIM))),
            wscl=wscl[None, :],
        ))

    return nc, in_maps


def _assemble(res, B=16, per=2):
    p = PATCH
    hh = IMG // p
    out = np.zeros((B, CIN, IMG, IMG), np.float32)
    for c in range(B // per):
        zo = res.results[c]["zout"]  # [512, 768]
        for i in range(per):
            zi = zo[i * 256:(i + 1) * 256]
            out[c * per + i] = zi.reshape(hh, hh, CIN, p, p).transpose(2, 0, 3, 1, 4).reshape(CIN, IMG, IMG)
    return out


def kernel(**inputs):
    nc, in_maps = _prepare(inputs)
    res = run_bass_kernel_spmd(nc, in_maps, list(range(len(in_maps))), trace=False)
    return _assemble(res)
